# revision 1
# baseline (speedup 1.0000x reference)
"""CGConvBlock (3x CGConv + MLP/BatchNorm + graph LayerNorm) on 8 Trainium2 cores.

Sharding: nodes by graph (4 graphs/core, per-graph padded to GPAD rows);
edges by destination core (sorted by padded dst). Per layer:
  - gather x[dst], x[src] (bf16, feature-major) via transposed dma_gather
  - gate/core pre-acts: z-chunk-stationary bf16 matmuls -> PSUM [edge, 256]
  - msg = softplus(core) * sigmoid(gate) via Exp/Ln table + DVE reciprocal
  - scatter-add: matmul(lhsT=msg[e,c], rhs=onehot[e,n]) accumulated per
    128-node block in PSUM -> [c, n] aggregation
  - MLP with global BatchNorm (stats allreduced), residuals in fp32
  - per-graph LayerNorm with static segments (graph-padded layout)
  - AllGather of the bf16 node table for the next layer's gathers
"""
import sys

sys.path.insert(0, "/opt/trn_rl_repo")

import numpy as np
import ml_dtypes

N = 20000
E = 640000
C = 128
D = 64
H = 512
L = 3
G = 32
EPS = 1e-5
NCORES = 8
GPG = G // NCORES  # graphs per core = 4
SG = 6             # edge tiles per activation supergroup
NCH = 512          # node free-dim chunk for MLP/LN

BF16 = ml_dtypes.bfloat16


def _wrap_idx(idx):
    """[n] int -> [16, n//16] int16 in the gather engine's wrapped layout
    (replicated to 128 partitions on device)."""
    n = idx.shape[0]
    assert n % 16 == 0
    return np.ascontiguousarray(idx.reshape(n // 16, 16).T.astype(np.int16))


def _preprocess(x, node_batch, edge_index, edge_attr):
    nb = np.asarray(node_batch)
    ei = np.asarray(edge_index)
    NN = nb.shape[0]
    counts = np.bincount(nb, minlength=G)
    gstart = np.concatenate([[0], np.cumsum(counts)[:-1]])
    GPAD = max(128, int(np.ceil(counts.max() / 128)) * 128)
    NPAD = GPG * GPAD
    NB = NPAD // 128
    PTOT = NCORES * NPAD
    assert PTOT < 32768

    g_of = nb  # nodes sorted by graph
    core_of_node = g_of // GPG
    slot = (g_of % GPG) * GPAD + (np.arange(NN) - gstart[g_of])
    pad_slot = core_of_node * NPAD + slot  # global padded index

    src, dst = ei[0], ei[1]
    core_of_edge = core_of_node[dst]
    dst_local = pad_slot[dst] - core_of_edge * NPAD

    # per (core, block) edge lists
    order = np.lexsort((dst_local, core_of_edge))
    src_s, dst_s = src[order], dst_local[order]
    core_s = core_of_edge[order]
    blk_s = dst_s // 128
    # counts[core, block]
    cb = np.zeros((NCORES, NB), np.int64)
    np.add.at(cb, (core_s, blk_s), 1)
    T_b = [int(np.ceil(cb[:, b].max() / 128)) for b in range(NB)]
    TT = sum(T_b)
    EP = TT * 128

    core_edge_start = np.concatenate([[0], np.cumsum(np.bincount(core_s, minlength=NCORES))])
    per_core = []
    x_np = np.asarray(x)
    ea = np.asarray(edge_attr)

    for c in range(NCORES):
        lo, hi = core_edge_start[c], core_edge_start[c + 1]
        srcc, dstc, blkc = src_s[lo:hi], dst_s[lo:hi], blk_s[lo:hi]
        eidc = order[lo:hi]
        # slot edges into padded per-block tile space
        src_pad = np.zeros(EP, np.int64)
        dstrel = np.full(EP, -1.0, np.float32)
        attr_sel = np.zeros(EP, np.int64)
        attr_valid = np.zeros(EP, bool)
        off = 0
        boff = np.concatenate([[0], np.cumsum(np.bincount(blkc, minlength=NB))])
        for b in range(NB):
            cnt = boff[b + 1] - boff[b]
            sl = slice(boff[b], boff[b + 1])
            src_pad[off:off + cnt] = pad_slot[srcc[sl]]
            dstrel[off:off + cnt] = (dstc[sl] - 128 * b).astype(np.float32)
            attr_sel[off:off + cnt] = eidc[sl]
            attr_valid[off:off + cnt] = True
            off += T_b[b] * 128
        assert off == EP
        dst_pad = np.zeros(EP, np.int64)
        m = dstrel >= 0
        dst_pad[m] = (dstrel[m] + 128 * np.repeat(np.arange(NB), np.array(T_b) * 128)[m]).astype(np.int64)

        attrT = np.zeros((65, EP), BF16)
        attrT[:D, attr_valid] = ea[attr_sel[attr_valid]].astype(BF16).T
        attrT[D:, :] = 0
        attrT[64, :] = BF16(1.0)

        xcn = np.zeros((C, NPAD), np.float32)
        mask = np.zeros((1, NPAD), np.float32)
        own = core_of_node == c
        xcn[:, slot[own]] = x_np[own].T
        mask[0, slot[own]] = 1.0
        xncbf = np.zeros((NPAD, C), BF16)
        xncbf[slot[own]] = x_np[own].astype(BF16)
        invcnt = (1.0 / (np.maximum(counts[c * GPG:(c + 1) * GPG], 1) * C)).astype(np.float32).reshape(1, GPG)

        per_core.append(dict(
            srcidx=_wrap_idx(src_pad),
            dstidx=_wrap_idx(dst_pad),
            dstrel=np.ascontiguousarray(dstrel.reshape(TT, 128).T),  # [128, TT]
            attrt=attrT,
            xcn=xcn,
            mask=np.ascontiguousarray(np.broadcast_to(mask, (128, NPAD))),
            invcnt=invcnt,
            xncbf=xncbf,
        ))

    meta = dict(GPAD=GPAD, NPAD=NPAD, NB=NB, PTOT=PTOT, T_b=T_b, TT=TT, EP=EP,
                pad_slot=pad_slot, counts=counts, NN=NN)
    return per_core, meta


def _prep_weights(Wf, bf, Ws, bs, W1, b1, g1, be1, W2, b2, lnw, lnb):
    wz = np.zeros((L, 3, 128, 2 * C), np.float32)
    for l in range(L):
        wcat = np.concatenate([-Wf[l].T, Ws[l].T], axis=1)  # [Z, 2C] (gate negated)
        wz[l, 0, :, :] = wcat[0:128]
        wz[l, 1, :, :] = wcat[128:256]
        wz[l, 2, :D, :] = wcat[256:320]
        wz[l, 2, 64, :] = np.concatenate([-bf[l], bs[l]])
    w1t = np.stack([np.stack([W1[l].T[:, 128 * k:128 * (k + 1)] for k in range(4)]) for l in range(L)])
    w2t = np.stack([np.stack([W2[l].T[128 * k:128 * (k + 1), :] for k in range(4)]) for l in range(L)])
    return dict(
        wz=wz.reshape(L * 3, 128, 2 * C).astype(BF16),
        w1t=w1t.reshape(L * 4, 128, 128).astype(BF16),
        w2t=w2t.reshape(L * 4, 128, 128).astype(BF16),
        b1c=np.ascontiguousarray(np.asarray(b1, np.float32).reshape(L, 4, 128).transpose(2, 0, 1).reshape(128, L * 4)),
        g1c=np.ascontiguousarray(np.asarray(g1, np.float32).reshape(L, 4, 128).transpose(2, 0, 1).reshape(128, L * 4)),
        be1c=np.ascontiguousarray(np.asarray(be1, np.float32).reshape(L, 4, 128).transpose(2, 0, 1).reshape(128, L * 4)),
        b2c=np.ascontiguousarray(np.asarray(b2, np.float32).T),     # [128, L]
        lnwr=np.asarray(lnw, np.float32).reshape(1, L * 128),
        lnbr=np.asarray(lnb, np.float32).reshape(1, L * 128),
    )


def _trace(meta, nlayers=L, use_cc=True, edge_only=False):
    from concourse import bacc, mybir
    import concourse.tile as tile

    # Force every activation onto the exp+ln table (index 6) so the ACT
    # table-load pass never alternates tables between Exp and Ln ops.
    import concourse.hw_specs as _hw
    if not hasattr(bacc, "_orig_get_act_tables"):
        bacc._orig_get_act_tables = bacc.get_activation_tables

        def _only_table6(arch):
            tabs = bacc._orig_get_act_tables(arch)
            out = {}
            for i, (name, funcs) in enumerate(tabs.items()):
                out[name] = funcs if name == "natural_log_exp_and_others" else set()
            return out

        bacc.get_activation_tables = _only_table6

    F32 = mybir.dt.float32
    B16 = mybir.dt.bfloat16
    I16 = mybir.dt.int16
    AF = mybir.ActivationFunctionType
    OP = mybir.AluOpType

    NPAD, NB, PTOT, TT, EP = meta["NPAD"], meta["NB"], meta["PTOT"], meta["TT"], meta["EP"]
    GPAD = meta["GPAD"]
    T_b = meta["T_b"]
    NNCH = (NPAD + NCH - 1) // NCH  # node chunks (NPAD % 512 may be 256)

    nc = bacc.Bacc("TRN2", target_bir_lowering=False, debug=False, num_devices=NCORES)

    def din(name, shape, dt):
        return nc.dram_tensor(name, shape, dt, kind="ExternalInput").ap()

    xcn_in = din("xcn", [C, NPAD], F32)
    xncbf_in = din("xncbf", [NPAD, C], B16)
    srcidx_in = din("srcidx", [16, TT * 8], I16)
    dstidx_in = din("dstidx", [16, TT * 8], I16)
    dstrel_in = din("dstrel", [128, TT], F32)
    attrt_in = din("attrt", [65, EP], B16)
    mask_in = din("mask", [128, NPAD], F32)
    invcnt_in = din("invcnt", [1, GPG], F32)
    iota_in = din("iota", [128, SG * 128], B16)
    ident_in = din("ident", [128, 128], F32)
    ones_in = din("ones", [1, NPAD], F32)
    wz_in = din("wz", [L * 3, 128, 2 * C], B16)
    w1t_in = din("w1t", [L * 4, 128, 128], B16)
    w2t_in = din("w2t", [L * 4, 128, 128], B16)
    b1c_in = din("b1c", [128, L * 4], F32)
    g1c_in = din("g1c", [128, L * 4], F32)
    be1c_in = din("be1c", [128, L * 4], F32)
    b2c_in = din("b2c", [128, L], F32)
    lnwr_in = din("lnwr", [1, L * 128], F32)
    lnbr_in = din("lnbr", [1, L * 128], F32)
    xout = nc.dram_tensor("xout", [NPAD, C], F32, kind="ExternalOutput").ap()

    with tile.TileContext(nc) as tc:
        with (
            tc.tile_pool(name="const", bufs=1) as cp,
            tc.tile_pool(name="xstate", bufs=2) as xp,
            tc.tile_pool(name="dram", bufs=1, space="DRAM") as dr,
        ):
            # ---- constants ----
            srcidx = cp.tile([128, TT * 8], I16)
            dstidx = cp.tile([128, TT * 8], I16)
            for r in range(8):
                nc.sync.dma_start(out=srcidx[16 * r:16 * (r + 1), :], in_=srcidx_in[:])
                nc.sync.dma_start(out=dstidx[16 * r:16 * (r + 1), :], in_=dstidx_in[:])
            dstrel = cp.tile([128, TT], F32)
            nc.sync.dma_start(out=dstrel[:], in_=dstrel_in[:])
            mask = cp.tile([128, NPAD], F32)
            nc.sync.dma_start(out=mask[:], in_=mask_in[:])
            invcnt = cp.tile([1, GPG], F32)
            nc.sync.dma_start(out=invcnt[:], in_=invcnt_in[:])
            iota = cp.tile([128, SG * 128], B16)
            nc.sync.dma_start(out=iota[:], in_=iota_in[:])
            ident = cp.tile([128, 128], F32)
            nc.sync.dma_start(out=ident[:], in_=ident_in[:])
            ones = cp.tile([1, NPAD], F32)
            nc.sync.dma_start(out=ones[:], in_=ones_in[:])
            wz = cp.tile([128, L * 3, 2 * C], B16)
            for i in range(L * 3):
                nc.sync.dma_start(out=wz[:, i, :], in_=wz_in[i])
            w1t = cp.tile([128, L * 4, 128], B16)
            w2t = cp.tile([128, L * 4, 128], B16)
            for i in range(L * 4):
                nc.sync.dma_start(out=w1t[:, i, :], in_=w1t_in[i])
                nc.sync.dma_start(out=w2t[:, i, :], in_=w2t_in[i])
            b1c = cp.tile([128, L * 4], F32)
            nc.sync.dma_start(out=b1c[:], in_=b1c_in[:])
            g1c = cp.tile([128, L * 4], F32)
            nc.sync.dma_start(out=g1c[:], in_=g1c_in[:])
            be1c = cp.tile([128, L * 4], F32)
            nc.sync.dma_start(out=be1c[:], in_=be1c_in[:])
            b2c = cp.tile([128, L], F32)
            nc.sync.dma_start(out=b2c[:], in_=b2c_in[:])
            lnwr = cp.tile([1, L * 128], F32)
            nc.sync.dma_start(out=lnwr[:], in_=lnwr_in[:])
            lnbr = cp.tile([1, L * 128], F32)
            nc.sync.dma_start(out=lnbr[:], in_=lnbr_in[:])
            epsc = cp.tile([128, 1], F32)
            nc.gpsimd.memset(epsc[:], EPS)

            x_fp = []
            for i in range(GPG):
                xfc = xp.tile([C, GPAD], F32, tag=f"xf{i}", name=f"xf{i}_0")
                nc.sync.dma_start(out=xfc[:], in_=xcn_in[:, i * GPAD:(i + 1) * GPAD])
                x_fp.append(xfc)

            tab0_b = dr.tile([NPAD, C], B16, tag="tab0b")
            nc.sync.dma_start(out=tab0_b[:], in_=xncbf_in[:])
            xtab_start = dr.tile([PTOT, C], B16, tag="xtab0", addr_space="Shared")
            if use_cc:
                nc.gpsimd.collective_compute(
                    "AllGather", OP.bypass, replica_groups=[list(range(NCORES))],
                    ins=[tab0_b[:].opt()], outs=[xtab_start[:].opt()])
            else:
                nc.sync.dma_start(out=xtab_start[0:NPAD, :], in_=tab0_b[:])
            agouts = []
            agins = []
            for l in range(nlayers):
                # ---------------- edge phase ----------------
                tab = xtab_start[:] if l == 0 else agouts[l - 1][:]
                dtab = xncbf_in if l == 0 else agins[l - 1][:]
                x2_fp = [xp.tile([C, GPAD], F32, tag=f"x2f{i}", bufs=2, name=f"x2f{i}_{l}") for i in range(GPG)]
                x2_bf = [xp.tile([C, GPAD], B16, tag=f"x2b{i}", bufs=2, name=f"x2b{i}_{l}") for i in range(GPG)]
                with (
                    tc.tile_pool(name="egath", bufs=2) as gp,
                    tc.tile_pool(name="eact", bufs=2) as ep,
                    tc.tile_pool(name="epsum", bufs=2, space="PSUM") as pp,
                    tc.tile_pool(name="aggpsum", bufs=2, space="PSUM") as ap_,
                ):
                    toff = 0
                    for b in range(NB):
                        tb = T_b[b]
                        ci, co = 128 * b // GPAD, 128 * b % GPAD
                        if tb == 0:
                            nc.vector.tensor_copy(x2_fp[ci][:, co:co + 128], x_fp[ci][:, co:co + 128])
                            nc.vector.tensor_copy(x2_bf[ci][:, co:co + 128], x_fp[ci][:, co:co + 128])
                            continue
                        ni = tb * 128
                        zdst = gp.tile([128, 1, ni], B16, tag="zdst")
                        nc.gpsimd.dma_gather(zdst[:], dtab, dstidx[:, toff * 8:(toff + tb) * 8],
                                             num_idxs=ni, num_idxs_reg=ni, elem_size=C, transpose=True, single_packet=False)
                        zsrc = gp.tile([128, 1, ni], B16, tag="zsrc")
                        nc.gpsimd.dma_gather(zsrc[:], tab, srcidx[:, toff * 8:(toff + tb) * 8],
                                             num_idxs=ni, num_idxs_reg=ni, elem_size=C, transpose=True, single_packet=False)
                        attr = gp.tile([65, ni], B16, tag="attr")
                        nc.sync.dma_start(out=attr[:], in_=attrt_in[:, toff * 128:toff * 128 + ni])
                        agg = ap_.tile([C, 128], F32, tag="agg", space="PSUM")
                        t0 = 0
                        while t0 < tb:
                            sgn = min(SG, tb - t0)
                            pre = pp.tile([128, SG, 2 * C], F32, tag="pre", space="PSUM")
                            for t in range(t0, t0 + sgn):
                                s = t - t0
                                nc.tensor.matmul(out=pre[:, s, :], lhsT=zdst[:, 0, t * 128:(t + 1) * 128],
                                                 rhs=wz[:, 3 * l + 0, :], start=True, stop=False)
                                nc.tensor.matmul(out=pre[:, s, :], lhsT=zsrc[:, 0, t * 128:(t + 1) * 128],
                                                 rhs=wz[:, 3 * l + 1, :], start=False, stop=False)
                                nc.tensor.matmul(out=pre[:, s, :], lhsT=attr[0:65, t * 128:(t + 1) * 128],
                                                 rhs=wz[0:65, 3 * l + 2, :], start=False, stop=True)
                            uv = ep.tile([128, SG, 256], F32, tag="uv")
                            sp = ep.tile([128, SG, 128], F32, tag="sp")
                            r = ep.tile([128, SG, 128], F32, tag="r")
                            msg = ep.tile([128, SG, 128], B16, tag="msg")
                            oh = ep.tile([128, SG, 128], B16, tag="oh")
                            nc.scalar.activation(uv[:, :sgn, :], pre[:, :sgn, :], AF.Exp)
                            nc.scalar.activation(sp[:, :sgn, :], uv[:, :sgn, C:2 * C], AF.Ln, bias=1.0)
                            nc.gpsimd.tensor_scalar_add(uv[:, :sgn, 0:C], uv[:, :sgn, 0:C], 1.0)
                            nc.vector.reciprocal_approx_fast(out=r[:, :sgn, :], in_=uv[:, :sgn, 0:C])
                            nc.vector.tensor_tensor(out=msg[:, :sgn, :], in0=sp[:, :sgn, :], in1=r[:, :sgn, :], op=OP.mult)
                            for t in range(t0, t0 + sgn):
                                s_ = t - t0
                                nc.vector.tensor_scalar(
                                    out=oh[:, s_, :], in0=iota[:, 0:128],
                                    scalar1=dstrel[:, toff + t:toff + t + 1], scalar2=None,
                                    op0=OP.is_equal)
                            for t in range(t0, t0 + sgn):
                                s = t - t0
                                nc.tensor.matmul(out=agg[:], lhsT=msg[:, s, :], rhs=oh[:, s, :],
                                                 start=(t == 0), stop=(t == tb - 1))
                            t0 += sgn
                        nc.vector.tensor_tensor(out=x2_fp[ci][:, co:co + 128],
                                                in0=x_fp[ci][:, co:co + 128], in1=agg[:], op=OP.add)
                        nc.vector.tensor_copy(x2_bf[ci][:, co:co + 128], x2_fp[ci][:, co:co + 128])
                        toff += tb

                if edge_only:
                    with tc.tile_pool(name="dbg", bufs=2, space="PSUM") as dbp:
                        for b in range(NB):
                            tp = dbp.tile([128, 128], F32, tag="dtp", space="PSUM")
                            nc.tensor.transpose(out=tp[:], in_=x2_fp[:, 128 * b:128 * (b + 1)], identity=ident[:])
                            xo = xp.tile([128, 128], F32, tag="dxo", bufs=2)
                            nc.scalar.activation(xo[:], tp[:], AF.Copy)
                            nc.sync.dma_start(out=xout[128 * b:128 * (b + 1), :], in_=xo[:])
                    break
                # ---------------- node phase (graph-major) ----------------
                with (
                    tc.tile_pool(name="nsb", bufs=1) as np_,
                    tc.tile_pool(name="nwork", bufs=3) as nw,
                    tc.tile_pool(name="npsum", bufs=1, space="PSUM") as npp,
                ):
                    h_bf = np_.tile([128, 4, NPAD], B16)
                    s1p = np_.tile([128, 4, GPG], F32)
                    s2p = np_.tile([128, 4, GPG], F32)
                    for g in range(GPG):
                        glo = g * GPAD
                        for k in range(4):
                            hp = npp.tile([128, GPAD], F32, tag="hpxp", space="PSUM", bufs=2)
                            for mlo in range(0, GPAD, NCH):
                                w = min(NCH, GPAD - mlo)
                                nc.tensor.matmul(out=hp[:, mlo:mlo + w], lhsT=w1t[:, 4 * l + k, :],
                                                 rhs=x2_bf[g][:, mlo:mlo + w], start=True, stop=True)
                            nc.vector.tensor_scalar(out=hp[:], in0=hp[:],
                                                    scalar1=b1c[:, 4 * l + k:4 * l + k + 1], scalar2=None, op0=OP.add)
                            nc.vector.tensor_tensor(out=h_bf[:, k, glo:glo + GPAD], in0=hp[:],
                                                    in1=mask[:, glo:glo + GPAD], op=OP.mult)
                            nc.vector.tensor_reduce(out=s1p[:, k, g:g + 1], in_=h_bf[:, k, glo:glo + GPAD],
                                                    axis=mybir.AxisListType.X, op=OP.add)
                            sq = nw.tile([128, GPAD], F32, tag="sq")
                            nc.scalar.activation(sq[:], h_bf[:, k, glo:glo + GPAD], AF.Square,
                                                 accum_out=s2p[:, k, g:g + 1])
                    bnstat = np_.tile([128, 8], F32)
                    nc.vector.tensor_reduce(out=bnstat[:, 0:4], in_=s1p[:], axis=mybir.AxisListType.X, op=OP.add)
                    nc.vector.tensor_reduce(out=bnstat[:, 4:8], in_=s2p[:], axis=mybir.AxisListType.X, op=OP.add)
                    bnin = dr.tile([128, 8], F32, tag="bnin", bufs=2)
                    bnout = dr.tile([128, 8], F32, tag="bnout", bufs=2, addr_space="Shared")
                    nc.sync.dma_start(out=bnin[:], in_=bnstat[:])
                    bns = np_.tile([128, 8], F32)
                    if use_cc:
                        nc.gpsimd.collective_compute(
                            "AllReduce", OP.add, replica_groups=[list(range(NCORES))],
                            ins=[bnin[:].opt()], outs=[bnout[:].opt()])
                        nc.sync.dma_start(out=bns[:], in_=bnout[:])
                    else:
                        nc.vector.tensor_scalar(out=bns[:], in0=bnstat[:], scalar1=float(NCORES), scalar2=None, op0=OP.mult)
                    mean = np_.tile([128, 4], F32)
                    nc.vector.tensor_scalar(out=mean[:], in0=bns[:, 0:4], scalar1=1.0 / meta["NN"], scalar2=None, op0=OP.mult)
                    var = np_.tile([128, 4], F32)
                    nc.vector.tensor_scalar(out=var[:], in0=bns[:, 4:8], scalar1=1.0 / meta["NN"], scalar2=None, op0=OP.mult)
                    msq = np_.tile([128, 4], F32)
                    nc.vector.tensor_tensor(out=msq[:], in0=mean[:], in1=mean[:], op=OP.mult)
                    nc.vector.tensor_tensor(out=var[:], in0=var[:], in1=msq[:], op=OP.subtract)
                    rstd = np_.tile([128, 4], F32)
                    nc.scalar.activation(rstd[:], var[:], AF.Ln, bias=epsc[:])
                    nc.scalar.activation(rstd[:], rstd[:], AF.Exp, scale=-0.5)
                    a_bn = np_.tile([128, 4], F32)
                    nc.vector.tensor_tensor(out=a_bn[:], in0=rstd[:], in1=g1c[:, 4 * l:4 * l + 4], op=OP.mult)
                    b_bn = np_.tile([128, 4], F32)
                    nc.vector.tensor_tensor(out=b_bn[:], in0=mean[:], in1=a_bn[:], op=OP.mult)
                    nc.vector.tensor_tensor(out=b_bn[:], in0=be1c[:, 4 * l:4 * l + 4], in1=b_bn[:], op=OP.subtract)

                    if l < nlayers - 1:
                        agin = dr.tile([NPAD, C], B16, tag="agin", bufs=2)
                        agout = dr.tile([PTOT, C], B16, tag="agout", bufs=2, addr_space="Shared")
                        agins.append(agin)
                    y_fp = [xp.tile([C, GPAD], F32, tag=f"xf{i}", bufs=2, name=f"yf{i}_{l}") for i in range(GPG)]
                    for g in range(GPG):
                        glo = g * GPAD
                        xpp = npp.tile([128, GPAD], F32, tag="hpxp", space="PSUM", bufs=2)
                        for k in range(4):
                            hn = nw.tile([128, GPAD], B16, tag="hn")
                            nc.scalar.activation(hn[:], h_bf[:, k, glo:glo + GPAD], AF.Relu,
                                                 scale=a_bn[:, k:k + 1], bias=b_bn[:, k:k + 1])
                            for mlo in range(0, GPAD, NCH):
                                w = min(NCH, GPAD - mlo)
                                nc.tensor.matmul(out=xpp[:, mlo:mlo + w], lhsT=w2t[:, 4 * l + k, :],
                                                 rhs=hn[:, mlo:mlo + w], start=(k == 0), stop=(k == 3))
                        t1 = nw.tile([128, GPAD], F32, tag="t1n")
                        nc.vector.tensor_scalar(out=t1[:], in0=xpp[:],
                                                scalar1=b2c[:, l:l + 1], scalar2=None, op0=OP.add)
                        nc.vector.tensor_tensor(out=t1[:], in0=t1[:], in1=x2_fp[g][:], op=OP.add)
                        x3g = nw.tile([128, GPAD], F32, tag="x3g", bufs=2)
                        nc.vector.tensor_tensor(out=x3g[:], in0=t1[:], in1=mask[:, glo:glo + GPAD], op=OP.mult)
                        # LN stats for this graph only
                        lnp = np_.tile([128, 2], F32, tag="lnp", bufs=2)
                        nc.vector.tensor_reduce(out=lnp[:, 0:1], in_=x3g[:], axis=mybir.AxisListType.X, op=OP.add)
                        sqg = nw.tile([128, GPAD], F32, tag="sqg")
                        nc.scalar.activation(sqg[:], x3g[:], AF.Square, accum_out=lnp[:, 1:2])
                        lnt = np_.tile([1, 2], F32, tag="lnt", bufs=2)
                        nc.gpsimd.tensor_reduce(out=lnt[:], in_=lnp[:], axis=mybir.AxisListType.C, op=OP.add)
                        mv = np_.tile([1, 2], F32, tag="mv", bufs=2)
                        nc.vector.tensor_scalar(out=mv[:], in0=lnt[:], scalar1=invcnt[:, g:g + 1], scalar2=None, op0=OP.mult)
                        m2g = np_.tile([1, 1], F32, tag="m2g", bufs=2)
                        nc.vector.tensor_tensor(out=m2g[:], in0=mv[:, 0:1], in1=mv[:, 0:1], op=OP.mult)
                        vgg = np_.tile([1, 1], F32, tag="vgg", bufs=2)
                        nc.vector.tensor_tensor(out=vgg[:], in0=mv[:, 1:2], in1=m2g[:], op=OP.subtract)
                        rgg = np_.tile([1, 1], F32, tag="rgg", bufs=2)
                        nc.scalar.activation(rgg[:], vgg[:], AF.Ln, bias=epsc[0:1, :])
                        nc.scalar.activation(rgg[:], rgg[:], AF.Exp, scale=-0.5)
                        bgg = np_.tile([1, 1], F32, tag="bgg", bufs=2)
                        nc.vector.tensor_tensor(out=bgg[:], in0=mv[:, 0:1], in1=rgg[:], op=OP.mult)
                        nc.vector.tensor_scalar(out=bgg[:], in0=bgg[:], scalar1=-1.0, scalar2=None, op0=OP.mult)
                        arow = np_.tile([1, GPAD], F32, tag="arow", bufs=2)
                        brow = np_.tile([1, GPAD], F32, tag="brow", bufs=2)
                        nc.vector.tensor_scalar(out=arow[:], in0=ones[:, glo:glo + GPAD],
                                                scalar1=rgg[:], scalar2=None, op0=OP.mult)
                        nc.vector.tensor_scalar(out=brow[:], in0=ones[:, glo:glo + GPAD],
                                                scalar1=bgg[:], scalar2=None, op0=OP.mult)
                        for mlo in range(0, GPAD, NCH):
                            w = min(NCH, GPAD - mlo)
                            A = npp.tile([128, NCH], F32, tag="A", space="PSUM")
                            B = npp.tile([128, NCH], F32, tag="B", space="PSUM")
                            nc.tensor.matmul(out=A[:, :w], lhsT=lnwr[:, 128 * l:128 * (l + 1)],
                                             rhs=arow[:, mlo:mlo + w], start=True, stop=True)
                            nc.tensor.matmul(out=B[:, :w], lhsT=lnwr[:, 128 * l:128 * (l + 1)],
                                             rhs=brow[:, mlo:mlo + w], start=True, stop=False)
                            nc.tensor.matmul(out=B[:, :w], lhsT=lnbr[:, 128 * l:128 * (l + 1)],
                                             rhs=ones[:, glo + mlo:glo + mlo + w], start=False, stop=True)
                            t2 = nw.tile([128, NCH], F32, tag="t2n")
                            nc.vector.tensor_tensor(out=t2[:, :w], in0=x3g[:, mlo:mlo + w], in1=A[:, :w], op=OP.mult)
                            nc.vector.tensor_tensor(out=y_fp[g][:, mlo:mlo + w], in0=t2[:, :w], in1=B[:, :w], op=OP.add)
                        # transpose this graph's blocks and ship them
                        for bb in range(GPAD // 128):
                            gb = glo + 128 * bb
                            tp = npp.tile([128, 128], F32, tag="tp", space="PSUM", bufs=2)
                            nc.tensor.transpose(out=tp[:], in_=y_fp[g][:, 128 * bb:128 * bb + 128], identity=ident[:])
                            if l < nlayers - 1:
                                xnc = nw.tile([128, 128], B16, tag="xnc")
                                nc.scalar.activation(xnc[:], tp[:], AF.Copy)
                                nc.sync.dma_start(out=agin[gb:gb + 128, :], in_=xnc[:])
                            else:
                                xnc32 = nw.tile([128, 128], F32, tag="xnc32")
                                nc.scalar.activation(xnc32[:], tp[:], AF.Copy)
                                nc.sync.dma_start(out=xout[gb:gb + 128, :], in_=xnc32[:])
                    if l < nlayers - 1:
                        if use_cc:
                            nc.gpsimd.collective_compute(
                                "AllGather", OP.bypass, replica_groups=[list(range(NCORES))],
                                ins=[agin[:].opt()], outs=[agout[:].opt()])
                            agouts.append(agout)
                        else:
                            agouts.append(agout)
                            nc.sync.dma_start(out=agout[0:NPAD, :], in_=agin[:])
                x_fp = y_fp

    nc.finalize()
    return nc


_CACHE = {}


def kernel(x, node_batch, edge_index, edge_attr,
           Wf, bf, Ws, bs, W1, b1, g1, be1, W2, b2, lnw, lnb):
    from concourse.bass_utils import run_bass_kernel_spmd

    per_core, meta = _preprocess(x, node_batch, edge_index, edge_attr)
    wd = _prep_weights(Wf, bf, Ws, bs, W1, b1, g1, be1, W2, b2, lnw, lnb)
    key = (meta["NPAD"], meta["NN"], tuple(meta["T_b"]))
    if key not in _CACHE:
        _CACHE[key] = _trace(meta)
    nc = _CACHE[key]

    iota = np.ascontiguousarray(
        np.broadcast_to(np.arange(128, dtype=np.float32), (SG, 128, 128)).transpose(1, 0, 2)
        .reshape(128, SG * 128)).astype(BF16)
    ident = np.eye(128, dtype=np.float32)
    ones = np.ones((1, meta["NPAD"]), np.float32)
    in_maps = []
    for c in range(NCORES):
        m = dict(per_core[c])
        m.update(wd)
        m.update(iota=iota, ident=ident, ones=ones)
        in_maps.append(m)
    res = run_bass_kernel_spmd(nc, in_maps, list(range(NCORES)))

    pad_slot = meta["pad_slot"]
    NPAD = meta["NPAD"]
    out = np.zeros((meta["NN"], C), np.float32)
    for c in range(NCORES):
        own = (pad_slot >= c * NPAD) & (pad_slot < (c + 1) * NPAD)
        out[own] = res.results[c]["xout"][pad_slot[own] - c * NPAD]
    return out



# revision 32
# speedup vs baseline: 1.2601x; 1.2601x over previous
"""CGConvBlock (3x CGConv + MLP/BatchNorm + graph LayerNorm) on 8 Trainium2 cores.

Sharding: nodes by graph (4 graphs/core, per-graph padded to GPAD rows);
edges by destination core (sorted by padded dst). Per layer:
  - per-block U matmul precomputes the dst-side preacts for all 128 dst
    nodes of the block; a host-built fp8 one-hot (oh_dst[n,e]) broadcasts
    them to edges on the PE (replaces the dst DMA gather entirely)
  - x[src] gathered bf16 via transposed dma_gather from the allgathered table
  - gate/core pre-acts accumulate in PSUM [edge, 256]
  - msg = softplus(core) * sigmoid(gate): ACT Exp/Ln + Pool (1+u) +
    DVE reciprocal + DVE mult, bf16 tail
  - scatter-add: matmul(lhsT=msg[e,c], rhs=oh_scat[e,n] fp8) -> [c, n] in PSUM
  - MLP with global BatchNorm (stats allreduced); BN bias/scale folded so the
    h pass needs no mask; relu via DVE add+max with a_bn folded into W2
  - per-graph LayerNorm applied as one affine_mul_reduce per graph
  - AllGather of the bf16 node table for the next layer's gathers
"""
import sys

sys.path.insert(0, "/opt/trn_rl_repo")

import numpy as np
import ml_dtypes

N = 20000
E = 640000
C = 128
D = 64
H = 512
L = 3
G = 32
EPS = 1e-5
NCORES = 8
GPG = G // NCORES  # graphs per core = 4
SG = 6             # edge tiles per activation supergroup
SPL = 30           # edge tiles per gather split

BF16 = ml_dtypes.bfloat16
F8 = ml_dtypes.float8_e4m3


def _wrap_idx(idx):
    """[n] int -> [16, n//16] int16 in the gather engine's wrapped layout
    (replicated to 128 partitions on device)."""
    n = idx.shape[0]
    assert n % 16 == 0
    return np.ascontiguousarray(idx.reshape(n // 16, 16).T.astype(np.int16))


def _splits(T):
    nsp = (T + SPL - 1) // SPL
    base = T // nsp
    rem = T % nsp
    out = []
    t0 = 0
    for i in range(nsp):
        seg = base + (1 if i < rem else 0)
        out.append((t0, seg))
        t0 += seg
    return out


def _preprocess(x, node_batch, edge_index, edge_attr):
    nb = np.asarray(node_batch)
    ei = np.asarray(edge_index)
    NN = nb.shape[0]
    counts = np.bincount(nb, minlength=G)
    gstart = np.concatenate([[0], np.cumsum(counts)[:-1]])
    GPAD = max(128, int(np.ceil(counts.max() / 128)) * 128)
    NPAD = GPG * GPAD
    NB = NPAD // 128
    PTOT = NCORES * NPAD
    assert PTOT < 32768

    g_of = nb  # nodes sorted by graph
    core_of_node = g_of // GPG
    # LPT-balance nodes across each graph's blocks by in-degree so per-block
    # edge counts (and the shared tile padding) equalize across cores
    indeg = np.bincount(ei[1], minlength=NN)
    nblk_g = GPAD // 128
    within = np.zeros(NN, np.int64)
    for g in range(G):
        idx = np.nonzero(nb == g)[0]
        order_d = idx[np.argsort(-indeg[idx], kind="stable")]
        load = np.zeros(nblk_g)
        fill = np.zeros(nblk_g, np.int64)
        pos = np.empty(len(idx), np.int64)
        for i, node in enumerate(order_d):
            b = int(np.argmin(load + np.where(fill >= 128, 1e18, 0)))
            pos[i] = b * 128 + fill[b]
            fill[b] += 1
            load[b] += indeg[node]
        within[order_d] = pos
    slot = (g_of % GPG) * GPAD + within
    pad_slot = core_of_node * NPAD + slot  # global padded index

    src, dst = ei[0], ei[1]
    core_of_edge = core_of_node[dst]
    dst_local = pad_slot[dst] - core_of_edge * NPAD

    # per (core, block) edge lists
    order = np.lexsort((dst_local, core_of_edge))
    src_s, dst_s = src[order], dst_local[order]
    core_s = core_of_edge[order]
    blk_s = dst_s // 128
    cb = np.zeros((NCORES, NB), np.int64)
    np.add.at(cb, (core_s, blk_s), 1)
    T_b = [int(np.ceil(cb[:, b].max() / 128)) for b in range(NB)]
    TT = sum(T_b)
    EP = TT * 128

    core_edge_start = np.concatenate([[0], np.cumsum(np.bincount(core_s, minlength=NCORES))])
    per_core = []
    x_np = np.asarray(x)
    ea = np.asarray(edge_attr)

    for c in range(NCORES):
        lo, hi = core_edge_start[c], core_edge_start[c + 1]
        srcc, dstc, blkc = src_s[lo:hi], dst_s[lo:hi], blk_s[lo:hi]
        eidc = order[lo:hi]
        # slot edges into padded per-block tile space
        src_pad = np.zeros(EP, np.int64)
        dstrel = np.full(EP, -1, np.int64)
        attr_sel = np.zeros(EP, np.int64)
        attr_valid = np.zeros(EP, bool)
        off = 0
        boff = np.concatenate([[0], np.cumsum(np.bincount(blkc, minlength=NB))])
        for b in range(NB):
            cnt = boff[b + 1] - boff[b]
            sl = slice(boff[b], boff[b + 1])
            src_pad[off:off + cnt] = pad_slot[srcc[sl]]
            dstrel[off:off + cnt] = dstc[sl] - 128 * b
            attr_sel[off:off + cnt] = eidc[sl]
            attr_valid[off:off + cnt] = True
            off += T_b[b] * 128
        assert off == EP

        attrT = np.zeros((65, EP), BF16)
        attrT[:D, attr_valid] = ea[attr_sel[attr_valid]].astype(BF16).T
        attrT[64, :] = BF16(1.0)

        # one-hots (fp8, exact 0/1)
        epos = np.arange(EP)
        valid = dstrel >= 0
        oh_dst = np.zeros((128, EP), F8)
        oh_dst[dstrel[valid], epos[valid]] = F8(1.0)
        # oh_scat[p, t*128 + n] = 1 iff edge (tile t, slot p) has dstrel == n
        oh_scat = np.zeros((128, EP), F8)
        t_of = epos // 128
        p_of = epos % 128
        oh_scat[p_of[valid], t_of[valid] * 128 + dstrel[valid]] = F8(1.0)
        # pack [ohd | ohs] per gather split so one DMA fetches both
        ohcat = np.zeros((128, 2 * EP), F8)
        toff = 0
        for b in range(NB):
            T = T_b[b]
            for (ts0, seg) in _splits(T):
                e0 = (toff + ts0) * 128
                ni = seg * 128
                ohcat[:, 2 * e0:2 * e0 + ni] = oh_dst[:, e0:e0 + ni]
                ohcat[:, 2 * e0 + ni:2 * e0 + 2 * ni] = oh_scat[:, e0:e0 + ni]
            toff += T

        xcn = np.zeros((C, NPAD), np.float32)
        mask = np.zeros((1, NPAD), np.float32)
        own = core_of_node == c
        xcn[:, slot[own]] = x_np[own].T
        mask[0, slot[own]] = 1.0
        xncbf = np.zeros((NPAD, C), BF16)
        xncbf[slot[own]] = x_np[own].astype(BF16)
        invcnt = (1.0 / (np.maximum(counts[c * GPG:(c + 1) * GPG], 1) * C)).astype(np.float32)

        per_core.append(dict(
            srcidx=np.ascontiguousarray(np.tile(_wrap_idx(src_pad), (8, 1))),
            attrt=attrT,
            ohdst=oh_dst,
            ohsct=oh_scat,
            xcn=xcn,
            xbfc=xcn.astype(BF16),
            mask=np.ascontiguousarray(np.broadcast_to(mask, (128, NPAD))),
            invcntb=np.ascontiguousarray(np.broadcast_to(invcnt.reshape(1, GPG), (128, GPG))),
            xncbf=xncbf,
        ))

    meta = dict(GPAD=GPAD, NPAD=NPAD, NB=NB, PTOT=PTOT, T_b=T_b, TT=TT, EP=EP,
                pad_slot=pad_slot, counts=counts, NN=NN)
    return per_core, meta


def _prep_weights(Wf, bf, Ws, bs, W1, b1, g1, be1, W2, b2, lnw, lnb):
    # z-part weight blocks: columns 0:128 gate (negated for exp(-g)), 128:256 core
    wdst = np.zeros((L, 128, 256), np.float32)
    wsrc = np.zeros((L, 128, 256), np.float32)
    wa = np.zeros((L, 65, 256), np.float32)
    for l in range(L):
        wdst[l, :, 0:128] = -Wf[l][:, 0:C].T
        wdst[l, :, 128:256] = Ws[l][:, 0:C].T
        wsrc[l, :, 0:128] = -Wf[l][:, C:2 * C].T
        wsrc[l, :, 128:256] = Ws[l][:, C:2 * C].T
        wa[l, :D, 0:128] = -Wf[l][:, 2 * C:].T
        wa[l, :D, 128:256] = Ws[l][:, 2 * C:].T
        wa[l, 64, 0:128] = -bf[l]
        wa[l, 64, 128:256] = bs[l]
    w1t = np.stack([np.stack([W1[l].T[:, 128 * k:128 * (k + 1)] for k in range(4)]) for l in range(L)])
    w2t = np.stack([np.stack([W2[l].T[128 * k:128 * (k + 1), :] for k in range(4)]) for l in range(L)])
    return dict(
        wdst=wdst.astype(BF16),
        wsrc=wsrc.astype(BF16),
        wa=wa.astype(BF16),
        w1t=w1t.reshape(L * 4, 128, 128).astype(BF16),
        w2t=w2t.reshape(L * 4, 128, 128).astype(BF16),
        b1c=np.ascontiguousarray(np.asarray(b1, np.float32).reshape(L, 4, 128).transpose(2, 0, 1).reshape(128, L * 4)),
        g1c=np.ascontiguousarray(np.asarray(g1, np.float32).reshape(L, 4, 128).transpose(2, 0, 1).reshape(128, L * 4)),
        be1c=np.ascontiguousarray(np.asarray(be1, np.float32).reshape(L, 4, 128).transpose(2, 0, 1).reshape(128, L * 4)),
        b2c=np.ascontiguousarray(np.asarray(b2, np.float32).T),      # [128, L]
        lnwc=np.ascontiguousarray(np.asarray(lnw, np.float32).T),    # [128, L]
        lnbc=np.ascontiguousarray(np.asarray(lnb, np.float32).T),    # [128, L]
    )


def _trace(meta, nlayers=L, use_cc=True, debug_stage=None):
    from concourse import bacc, mybir, bass_isa
    import concourse.tile as tile

    # Force every activation onto the exp+ln table (index 6) so the ACT
    # table-load pass never alternates tables between Exp and Ln ops.
    if not hasattr(bacc, "_orig_get_act_tables"):
        bacc._orig_get_act_tables = bacc.get_activation_tables

        def _only_table6(arch):
            tabs = bacc._orig_get_act_tables(arch)
            out = {}
            for name, funcs in tabs.items():
                out[name] = funcs if name == "natural_log_exp_and_others" else set()
            return out

        bacc.get_activation_tables = _only_table6

    F32 = mybir.dt.float32
    B16 = mybir.dt.bfloat16
    F8E4 = mybir.dt.float8e4
    I16 = mybir.dt.int16
    AF = mybir.ActivationFunctionType
    OP = mybir.AluOpType

    NPAD, NB, PTOT, TT, EP = meta["NPAD"], meta["NB"], meta["PTOT"], meta["TT"], meta["EP"]
    GPAD = meta["GPAD"]
    T_b = meta["T_b"]
    NN = meta["NN"]

    nc = bacc.Bacc("TRN2", target_bir_lowering=False, debug=False, num_devices=NCORES)

    def din(name, shape, dt):
        return nc.dram_tensor(name, shape, dt, kind="ExternalInput").ap()

    xcn_in = din("xcn", [C, NPAD], F32)
    xbfc_in = din("xbfc", [C, NPAD], B16)
    xncbf_in = din("xncbf", [NPAD, C], B16)
    srcidx_in = din("srcidx", [128, TT * 8], I16)
    attrt_in = din("attrt", [65, EP], B16)
    ohdst_in = din("ohdst", [128, EP], F8E4)
    ohsct_in = din("ohsct", [128, EP], F8E4)
    mask_in = din("mask", [128, NPAD], F32)
    invcntb_in = din("invcntb", [128, GPG], F32)
    ident_in = din("ident", [128, 128], F32)
    identb_in = din("identb", [128, 128], B16)
    wdst_in = din("wdst", [L, 128, 256], B16)
    wsrc_in = din("wsrc", [L, 128, 256], B16)
    wa_in = din("wa", [L, 65, 256], B16)
    w1t_in = din("w1t", [L * 4, 128, 128], B16)
    w2t_in = din("w2t", [L * 4, 128, 128], B16)
    b1c_in = din("b1c", [128, L * 4], F32)
    g1c_in = din("g1c", [128, L * 4], F32)
    be1c_in = din("be1c", [128, L * 4], F32)
    b2c_in = din("b2c", [128, L], F32)
    lnwc_in = din("lnwc", [128, L], F32)
    lnbc_in = din("lnbc", [128, L], F32)
    xout = nc.dram_tensor("xout", [NPAD, C], F32, kind="ExternalOutput").ap()

    # per-block gather splits
    def splits(T):
        nsp = (T + SPL - 1) // SPL
        base = T // nsp
        rem = T % nsp
        out = []
        t0 = 0
        for i in range(nsp):
            seg = base + (1 if i < rem else 0)
            out.append((t0, seg))
            t0 += seg
        return out

    with tile.TileContext(nc) as tc:
        with (
            tc.tile_pool(name="const", bufs=1) as cp,
            tc.tile_pool(name="xstate", bufs=2) as xp,
            tc.tile_pool(name="dram", bufs=1, space="DRAM") as dr,
        ):
            # ---- constants (critical-path loads first) ----
            tab0_b = dr.tile([NPAD, C], B16, tag="tab0b")
            nc.sync.dma_start(out=tab0_b[:], in_=xncbf_in[:])
            srcidx = cp.tile([128, TT * 8], I16)
            nc.sync.dma_start(out=srcidx[:], in_=srcidx_in[:])
            x_bf = xp.tile([C, NPAD], B16, tag="xbf", bufs=1, name="xbf")
            nc.sync.dma_start(out=x_bf[:], in_=xbfc_in[:])
            wdst = cp.tile([128, L, 256], B16)
            wsrc = cp.tile([128, L, 256], B16)
            wa = cp.tile([65, L, 256], B16)
            for l in range(L):
                nc.sync.dma_start(out=wdst[:, l, :], in_=wdst_in[l])
                nc.sync.dma_start(out=wsrc[:, l, :], in_=wsrc_in[l])
                nc.sync.dma_start(out=wa[:, l, :], in_=wa_in[l])
            xtab_start = dr.tile([PTOT, C], B16, tag="xtab0", addr_space="Shared")
            if use_cc:
                nc.gpsimd.collective_compute(
                    "AllGather", OP.bypass, replica_groups=[list(range(NCORES))],
                    ins=[tab0_b[:].opt()], outs=[xtab_start[:].opt()])
            else:
                nc.sync.dma_start(out=xtab_start[0:NPAD, :], in_=tab0_b[:])

            x_fp = xp.tile([C, NPAD], F32, tag="xfp", bufs=1, name="xfp")
            nc.scalar.dma_start(out=x_fp[:], in_=xcn_in[:])
            U_all = xp.tile([128, NB, 256], B16, tag="uall", bufs=1, name="uall")
            mask = cp.tile([128, NPAD], F32)
            nc.scalar.dma_start(out=mask[:], in_=mask_in[:])
            invcntb = cp.tile([128, GPG], F32)
            nc.scalar.dma_start(out=invcntb[:], in_=invcntb_in[:])
            ident = cp.tile([128, 128], F32)
            nc.scalar.dma_start(out=ident[:], in_=ident_in[:])
            identb = cp.tile([128, 128], B16)
            nc.scalar.dma_start(out=identb[:], in_=identb_in[:])
            w1t = cp.tile([128, L * 4, 128], B16)
            w2t = cp.tile([128, L * 4, 128], B16)
            for i in range(L * 4):
                nc.scalar.dma_start(out=w1t[:, i, :], in_=w1t_in[i])
                nc.scalar.dma_start(out=w2t[:, i, :], in_=w2t_in[i])
            b1c = cp.tile([128, L * 4], F32)
            nc.scalar.dma_start(out=b1c[:], in_=b1c_in[:])
            g1c = cp.tile([128, L * 4], F32)
            nc.scalar.dma_start(out=g1c[:], in_=g1c_in[:])
            be1c = cp.tile([128, L * 4], F32)
            nc.scalar.dma_start(out=be1c[:], in_=be1c_in[:])
            b2c = cp.tile([128, L], F32)
            nc.scalar.dma_start(out=b2c[:], in_=b2c_in[:])
            lnwc = cp.tile([128, L], F32)
            nc.scalar.dma_start(out=lnwc[:], in_=lnwc_in[:])
            lnbc = cp.tile([128, L], F32)
            nc.scalar.dma_start(out=lnbc[:], in_=lnbc_in[:])
            epsc = cp.tile([128, 1], F32)
            nc.gpsimd.memset(epsc[:], EPS)
            agouts = []
            agins = []
            for l in range(nlayers):
                tab = xtab_start[:] if l == 0 else agouts[l - 1][:]
                if l == 0:
                    # dst-side preacts per block (later layers fold this into
                    # the previous node phase, per graph)
                    with tc.tile_pool(name="upsum", bufs=2, space="PSUM") as up:
                        for b in range(NB):
                            ups = up.tile([128, 256], F32, tag="ups", space="PSUM")
                            nc.tensor.matmul(out=ups[:], lhsT=x_bf[:, 128 * b:128 * (b + 1)],
                                             rhs=wdst[:, l, :], start=True, stop=True)
                            nc.vector.tensor_scalar(out=U_all[:, b, :], in0=ups[:],
                                                    scalar1=1.0, scalar2=0.0, op0=OP.mult, op1=OP.add)
                # ---------------- edge phase ----------------
                x2_fp = xp.tile([C, NPAD], F32, tag="x2fp", bufs=1, name=f"x2fp_{l}")
                x2_bf = xp.tile([C, NPAD], B16, tag="x2bf", bufs=1, name=f"x2bf_{l}")
                with (
                    tc.tile_pool(name="egath", bufs=3) as gp,
                    tc.tile_pool(name="eact", bufs=2) as ep,
                    tc.tile_pool(name="epsum", bufs=2, space="PSUM") as pp,
                    tc.tile_pool(name="aggpsum", bufs=2, space="PSUM") as ap_,
                ):
                    toff = 0
                    for b in range(NB):
                        T = T_b[b]
                        bcol = slice(128 * b, 128 * (b + 1))
                        if T == 0:
                            nc.vector.tensor_copy(x2_fp[:, bcol], x_fp[:, bcol])
                            nc.vector.tensor_copy(x2_bf[:, bcol], x_fp[:, bcol])
                            continue
                        agg = ap_.tile([C, 128], F32, tag="agg", space="PSUM")
                        for (ts0, seg) in splits(T):
                            ni = seg * 128
                            e0 = (toff + ts0) * 128
                            zsrc = gp.tile([128, 1, ni], B16, tag="zsrc")
                            nc.gpsimd.dma_gather(zsrc[:], tab, srcidx[:, (toff + ts0) * 8:(toff + ts0 + seg) * 8],
                                                 num_idxs=ni, num_idxs_reg=ni, elem_size=C,
                                                 transpose=True, single_packet=False)
                            attr = gp.tile([65, ni], B16, tag="attr")
                            nc.sync.dma_start(out=attr[:], in_=attrt_in[:, e0:e0 + ni])
                            ohd = gp.tile([128, ni], F8E4, tag="ohd")
                            nc.sync.dma_start(out=ohd[:], in_=ohdst_in[:, e0:e0 + ni])
                            ohs = gp.tile([128, ni], F8E4, tag="ohs")
                            nc.sync.dma_start(out=ohs[:], in_=ohsct_in[:, e0:e0 + ni])
                            t0 = ts0
                            while t0 < ts0 + seg:
                                sgn = min(SG, ts0 + seg - t0)
                                pre = pp.tile([128, SG, 256], F32, tag="pre", space="PSUM")
                                for t in range(t0, t0 + sgn):
                                    s = t - t0
                                    esl = slice((t - ts0) * 128, (t - ts0 + 1) * 128)
                                    nc.tensor.matmul(out=pre[:, s, :], lhsT=ohd[:, esl],
                                                     rhs=U_all[:, b, :], start=True, stop=False)
                                    nc.tensor.matmul(out=pre[:, s, :], lhsT=zsrc[:, 0, esl],
                                                     rhs=wsrc[:, l, :], start=False, stop=False)
                                    nc.tensor.matmul(out=pre[:, s, :], lhsT=attr[0:65, esl],
                                                     rhs=wa[0:65, l, :], start=False, stop=True)
                                uv = ep.tile([128, SG, 256], B16, tag="uv")
                                t32 = ep.tile([128, SG, 128], F32, tag="t32")
                                sp = ep.tile([128, SG, 128], B16, tag="sp")
                                r32 = ep.tile([128, SG, 128], F32, tag="r32")
                                msg = ep.tile([128, SG, 128], B16, tag="msg")
                                nc.scalar.activation(uv[:, :sgn, :], pre[:, :sgn, :], AF.Exp)
                                nc.scalar.activation(sp[:, :sgn, :], uv[:, :sgn, C:2 * C], AF.Ln, bias=1.0)
                                nc.gpsimd.tensor_scalar_add(t32[:, :sgn, :], uv[:, :sgn, 0:C], 1.0)
                                nc.vector.reciprocal_approx_fast(out=r32[:, :sgn, :], in_=t32[:, :sgn, :])
                                nc.vector.tensor_tensor(out=msg[:, :sgn, :], in0=sp[:, :sgn, :],
                                                        in1=r32[:, :sgn, :], op=OP.mult)
                                for t in range(t0, t0 + sgn):
                                    s = t - t0
                                    nsl = slice((t - ts0) * 128, (t - ts0 + 1) * 128)
                                    nc.tensor.matmul(out=agg[:], lhsT=msg[:, s, :], rhs=ohs[:, nsl],
                                                     start=(t == 0), stop=(t == T - 1))
                                t0 += sgn
                        nc.vector.tensor_tensor(out=x2_fp[:, bcol], in0=x_fp[:, bcol], in1=agg[:], op=OP.add)
                        nc.vector.tensor_scalar(out=x2_bf[:, bcol], in0=x2_fp[:, bcol],
                                                scalar1=1.0, scalar2=0.0, op0=OP.mult, op1=OP.add)
                        toff += T

                if debug_stage == "x2":
                    with tc.tile_pool(name="dbg", bufs=2, space="PSUM") as dbp:
                        with tc.tile_pool(name="dbw", bufs=2) as dbw:
                            for b in range(NB):
                                tpd = dbp.tile([128, 128], F32, tag="dtp", space="PSUM")
                                nc.tensor.transpose(out=tpd[:], in_=x2_fp[:, 128 * b:128 * (b + 1)],
                                                    identity=ident[:])
                                xo = dbw.tile([128, 128], F32, tag="dxo")
                                nc.vector.tensor_copy(xo[:], tpd[:])
                                nc.sync.dma_start(out=xout[128 * b:128 * (b + 1), :], in_=xo[:])
                    break
                # ---------------- node phase ----------------
                with tc.tile_pool(name="nsb", bufs=1) as np_:
                    h_bf = np_.tile([128, 4, NPAD], B16)
                    s1p = np_.tile([128, 4, GPG], F32)
                    s2p = np_.tile([128, 4, GPG], F32)
                    with (
                        tc.tile_pool(name="hwork", bufs=2) as hw,
                        tc.tile_pool(name="hpsum", bufs=2, space="PSUM") as hp_,
                    ):
                        for g in range(GPG):
                            glo = g * GPAD
                            gsl = slice(glo, glo + GPAD)
                            for k in range(4):
                                hp = hp_.tile([128, GPAD], F32, tag="hp", space="PSUM")
                                for mlo in range(0, GPAD, 512):
                                    w = min(512, GPAD - mlo)
                                    nc.tensor.matmul(out=hp[:, mlo:mlo + w], lhsT=w1t[:, 4 * l + k, :],
                                                     rhs=x2_bf[:, glo + mlo:glo + mlo + w], start=True, stop=True)
                                # h (no bias; padded cols exactly 0) + rowsum on Pool
                                nc.vector.tensor_scalar(out=h_bf[:, k, gsl], in0=hp[:],
                                                        scalar1=1.0, scalar2=0.0, op0=OP.mult, op1=OP.add,
                                                        accum_out=s1p[:, k, g:g + 1])
                                junk = hw.tile([128, GPAD], B16, tag="junk")
                                nc.scalar.activation(junk[:], h_bf[:, k, gsl], AF.Square,
                                                     accum_out=s2p[:, k, g:g + 1])
                    bnstat = np_.tile([128, 8], F32)
                    nc.vector.tensor_reduce(out=bnstat[:, 0:4], in_=s1p[:], axis=mybir.AxisListType.X, op=OP.add)
                    nc.vector.tensor_reduce(out=bnstat[:, 4:8], in_=s2p[:], axis=mybir.AxisListType.X, op=OP.add)
                    bnin = dr.tile([128, 8], F32, tag="bnin", bufs=2)
                    bnout = dr.tile([128, 8], F32, tag="bnout", bufs=2, addr_space="Shared")
                    nc.gpsimd.dma_start(out=bnin[:], in_=bnstat[:])
                    bns = np_.tile([128, 8], F32)
                    if use_cc:
                        nc.gpsimd.collective_compute(
                            "AllReduce", OP.add, replica_groups=[list(range(NCORES))],
                            ins=[bnin[:].opt()], outs=[bnout[:].opt()])
                        nc.gpsimd.dma_start(out=bns[:], in_=bnout[:])
                    else:
                        nc.vector.tensor_scalar(out=bns[:], in0=bnstat[:], scalar1=float(NCORES),
                                                scalar2=None, op0=OP.mult)
                    ksl = slice(4 * l, 4 * l + 4)
                    mean_r = np_.tile([128, 4], F32)
                    nc.vector.tensor_scalar(out=mean_r[:], in0=bns[:, 0:4], scalar1=1.0 / NN,
                                            scalar2=None, op0=OP.mult)
                    var = np_.tile([128, 4], F32)
                    nc.vector.tensor_scalar(out=var[:], in0=bns[:, 4:8], scalar1=1.0 / NN,
                                            scalar2=None, op0=OP.mult)
                    msq = np_.tile([128, 4], F32)
                    nc.vector.tensor_tensor(out=msq[:], in0=mean_r[:], in1=mean_r[:], op=OP.mult)
                    nc.vector.tensor_tensor(out=var[:], in0=var[:], in1=msq[:], op=OP.subtract)
                    rstd = np_.tile([128, 4], F32)
                    nc.scalar.activation(rstd[:], var[:], AF.Ln, bias=epsc[:])
                    nc.scalar.activation(rstd[:], rstd[:], AF.Exp, scale=-0.5)
                    a_bn = np_.tile([128, 4], F32)
                    nc.vector.tensor_tensor(out=a_bn[:], in0=rstd[:], in1=g1c[:, ksl], op=OP.mult)
                    inva = np_.tile([128, 4], F32)
                    nc.vector.reciprocal_approx_fast(out=inva[:], in_=a_bn[:])
                    # q = be1/a + b1 - (mean_r + b1) = be1/a - mean_r
                    q = np_.tile([128, 4], F32)
                    nc.vector.tensor_tensor(out=q[:], in0=be1c[:, ksl], in1=inva[:], op=OP.mult)
                    nc.vector.tensor_tensor(out=q[:], in0=q[:], in1=mean_r[:], op=OP.subtract)
                    w2s = np_.tile([128, 4, 128], B16)
                    for k in range(4):
                        nc.vector.tensor_scalar(out=w2s[:, k, :], in0=w2t[:, 4 * l + k, :],
                                                scalar1=a_bn[:, k:k + 1], scalar2=0.0,
                                                op0=OP.mult, op1=OP.add)

                    if l < nlayers - 1:
                        agin = dr.tile([NPAD, C], B16, tag="agin", bufs=2)
                        if use_cc:
                            agout = dr.tile([PTOT, C], B16, tag="agout", bufs=2, addr_space="Shared")
                        else:
                            agout = dr.tile([PTOT, C], B16, tag="agout", bufs=2)
                        agins.append(agin)
                    y_fp = xp.tile([C, NPAD], F32, tag="xfp", bufs=1, name=f"yfp_{l}")
                    y_bf = xp.tile([C, NPAD], B16, tag="xbf", bufs=1, name=f"ybf_{l}")
                    with (
                        tc.tile_pool(name="xwork", bufs=2) as nw,
                        tc.tile_pool(name="xpsum", bufs=1, space="PSUM") as xpp_,
                        tc.tile_pool(name="tpsum", bufs=2, space="PSUM") as tp_,
                    ):
                        for g in range(GPG):
                            glo = g * GPAD
                            gsl = slice(glo, glo + GPAD)
                            xpp = xpp_.tile([128, GPAD], F32, tag="xpp", space="PSUM")
                            for k in range(4):
                                hn = nw.tile([128, GPAD], B16, tag="hn")
                                nc.vector.tensor_scalar(out=hn[:], in0=h_bf[:, k, gsl],
                                                        scalar1=q[:, k:k + 1], scalar2=0.0,
                                                        op0=OP.add, op1=OP.max)
                                for mlo in range(0, GPAD, 512):
                                    w = min(512, GPAD - mlo)
                                    nc.tensor.matmul(out=xpp[:, mlo:mlo + w], lhsT=w2s[:, k, :],
                                                     rhs=hn[:, mlo:mlo + w], start=(k == 0), stop=False)
                            for mlo in range(0, GPAD, 512):
                                w = min(512, GPAD - mlo)
                                nc.tensor.matmul(out=xpp[:, mlo:mlo + w], lhsT=identb[:],
                                                 rhs=x2_bf[:, glo + mlo:glo + mlo + w],
                                                 start=False, stop=True)
                            # x3 = (xpp + b2) * mask, with row-sums for LN
                            x3m = nw.tile([128, GPAD], F32, tag="x3m")
                            ls = np_.tile([128, 2], F32, tag="ls", bufs=2)
                            nc.vector.affine_mul_reduce(out=x3m[:], accum_out=ls[:, 0:1],
                                                        in0=xpp[:], in1=mask[:, gsl],
                                                        scale=1.0, bias=b2c[:, l:l + 1])
                            junk2 = nw.tile([128, GPAD], B16, tag="junk2")
                            nc.scalar.activation(junk2[:], x3m[:], AF.Square,
                                                 accum_out=ls[:, 1:2])
                            if debug_stage == "x3":
                                for bb in range(GPAD // 128):
                                    gb = glo + 128 * bb
                                    tpd = tp_.tile([128, 128], F32, tag="dtp", space="PSUM")
                                    nc.tensor.transpose(out=tpd[:], in_=x3m[:, 128 * bb:128 * (bb + 1)],
                                                        identity=ident[:])
                                    xo = nw.tile([128, 128], F32, tag="dxo")
                                    nc.vector.tensor_copy(xo[:], tpd[:])
                                    nc.sync.dma_start(out=xout[gb:gb + 128, :], in_=xo[:])
                                continue
                            lsr = np_.tile([128, 2], F32, tag="lsr", bufs=2)
                            nc.gpsimd.partition_all_reduce(lsr[:], ls[:], channels=128,
                                                           reduce_op=bass_isa.ReduceOp.add)
                            mv = np_.tile([128, 2], F32, tag="mv", bufs=2)
                            nc.vector.tensor_scalar(out=mv[:], in0=lsr[:], scalar1=invcntb[:, g:g + 1],
                                                    scalar2=None, op0=OP.mult)
                            if debug_stage == "lnstats":
                                nc.sync.dma_start(out=xout[glo:glo + 1, 0:2], in_=ls[0:1, :])
                                nc.sync.dma_start(out=xout[glo + 1:glo + 2, 0:2], in_=lsr[0:1, :])
                                nc.sync.dma_start(out=xout[glo + 2:glo + 3, 0:2], in_=mv[0:1, :])
                                continue
                            m2g = np_.tile([128, 1], F32, tag="m2g", bufs=2)
                            nc.vector.tensor_tensor(out=m2g[:], in0=mv[:, 0:1], in1=mv[:, 0:1], op=OP.mult)
                            vgg = np_.tile([128, 1], F32, tag="vgg", bufs=2)
                            nc.vector.tensor_tensor(out=vgg[:], in0=mv[:, 1:2], in1=m2g[:], op=OP.subtract)
                            rgg = np_.tile([128, 1], F32, tag="rgg", bufs=2)
                            nc.scalar.activation(rgg[:], vgg[:], AF.Ln, bias=epsc[:])
                            nc.scalar.activation(rgg[:], rgg[:], AF.Exp, scale=-0.5)
                            scal = np_.tile([128, 1], F32, tag="scal", bufs=2)
                            nc.vector.tensor_tensor(out=scal[:], in0=lnwc[:, l:l + 1], in1=rgg[:], op=OP.mult)
                            bias = np_.tile([128, 1], F32, tag="bias", bufs=2)
                            nc.vector.tensor_tensor(out=bias[:], in0=mv[:, 0:1], in1=scal[:], op=OP.mult)
                            nc.vector.tensor_tensor(out=bias[:], in0=lnbc[:, l:l + 1], in1=bias[:], op=OP.subtract)
                            # y = (x3m * scal + bias) * mask  (pads stay exactly 0)
                            jacc = np_.tile([128, 1], F32, tag="jacc", bufs=2)
                            nc.vector.affine_mul_reduce(out=y_fp[:, gsl], accum_out=jacc[:],
                                                        in0=x3m[:], in1=mask[:, gsl],
                                                        scale=scal[:], bias=bias[:])
                            nc.vector.tensor_scalar(out=y_bf[:, gsl], in0=y_fp[:, gsl],
                                                    scalar1=1.0, scalar2=0.0, op0=OP.mult, op1=OP.add)
                            # ship this graph's blocks: batched transposes,
                            # one copy, one DMA
                            NBG = GPAD // 128
                            if l < nlayers - 1:
                                xnc = nw.tile([128, NBG, 128], B16, tag="xnc")
                                for bb in range(NBG):
                                    gb = glo + 128 * bb
                                    tp = tp_.tile([128, 128], B16, tag="tp", space="PSUM")
                                    nc.tensor.transpose(out=tp[:], in_=y_bf[:, gb:gb + 128],
                                                        identity=identb[:])
                                    nc.vector.tensor_scalar(out=xnc[:, bb, :], in0=tp[:], scalar1=1.0,
                                                            scalar2=0.0, op0=OP.mult, op1=OP.add)
                                    nc.gpsimd.dma_start(out=agin[gb:gb + 128, :], in_=xnc[:, bb, :])
                            else:
                                xnc32 = nw.tile([128, NBG, 128], F32, tag="xnc32")
                                for bb in range(NBG):
                                    gb = glo + 128 * bb
                                    tp = tp_.tile([128, 128], F32, tag="tpf", space="PSUM")
                                    nc.tensor.transpose(out=tp[:], in_=y_fp[:, gb:gb + 128],
                                                        identity=ident[:])
                                    nc.vector.tensor_scalar(out=xnc32[:, bb, :], in0=tp[:], scalar1=1.0,
                                                            scalar2=0.0, op0=OP.mult, op1=OP.add)
                                    nc.gpsimd.dma_start(out=xout[gb:gb + 128, :], in_=xnc32[:, bb, :])
                            if l < nlayers - 1:
                                # next layer's dst-side preacts for this graph's blocks
                                for bb in range(GPAD // 128):
                                    gb = glo + 128 * bb
                                    b = gb // 128
                                    ups = tp_.tile([128, 256], F32, tag="ups", space="PSUM")
                                    nc.tensor.matmul(out=ups[:], lhsT=y_bf[:, gb:gb + 128],
                                                     rhs=wdst[:, l + 1, :], start=True, stop=True)
                                    nc.vector.tensor_scalar(out=U_all[:, b, :], in0=ups[:],
                                                            scalar1=1.0, scalar2=0.0, op0=OP.mult, op1=OP.add)
                                if not use_cc:
                                    nc.gpsimd.dma_start(out=agout[glo:glo + GPAD, :],
                                                      in_=agin[glo:glo + GPAD, :])
                    if l < nlayers - 1:
                        agouts.append(agout)
                        if use_cc:
                            nc.gpsimd.collective_compute(
                                "AllGather", OP.bypass, replica_groups=[list(range(NCORES))],
                                ins=[agin[:].opt()], outs=[agout[:].opt()])
                x_fp = y_fp
                x_bf = y_bf

    nc.finalize()
    return nc


_CACHE = {}


def kernel(x, node_batch, edge_index, edge_attr,
           Wf, bf, Ws, bs, W1, b1, g1, be1, W2, b2, lnw, lnb):
    from concourse.bass_utils import run_bass_kernel_spmd

    per_core, meta = _preprocess(x, node_batch, edge_index, edge_attr)
    wd = _prep_weights(Wf, bf, Ws, bs, W1, b1, g1, be1, W2, b2, lnw, lnb)
    key = (meta["NPAD"], meta["NN"], tuple(meta["T_b"]))
    if key not in _CACHE:
        _CACHE[key] = _trace(meta)
    nc = _CACHE[key]

    ident = np.eye(128, dtype=np.float32)
    identb = np.eye(128, dtype=np.float32).astype(BF16)
    in_maps = []
    for c in range(NCORES):
        m = dict(per_core[c])
        m.update(wd)
        m.update(ident=ident, identb=identb)
        in_maps.append(m)
    res = run_bass_kernel_spmd(nc, in_maps, list(range(NCORES)))

    pad_slot = meta["pad_slot"]
    NPAD = meta["NPAD"]
    out = np.zeros((meta["NN"], C), np.float32)
    for c in range(NCORES):
        own = (pad_slot >= c * NPAD) & (pad_slot < (c + 1) * NPAD)
        out[own] = res.results[c]["xout"][pad_slot[own] - c * NPAD]
    return out


# revision 46
# speedup vs baseline: 1.3792x; 1.0945x over previous
"""CGConvBlock (3x CGConv + MLP/BatchNorm + graph LayerNorm) on 8 Trainium2 cores.

Sharding: nodes by graph (4 graphs/core, per-graph padded to GPAD rows);
edges by destination core (sorted by padded dst). Per layer:
  - per-block U matmul precomputes the dst-side preacts for all 128 dst
    nodes of the block; a host-built fp8 one-hot (oh_dst[n,e]) broadcasts
    them to edges on the PE (replaces the dst DMA gather entirely)
  - x[src] gathered bf16 via transposed dma_gather from the allgathered table
  - gate/core pre-acts accumulate in PSUM [edge, 256]
  - msg = softplus(core) * sigmoid(gate): ACT Exp/Ln + Pool (1+u) +
    DVE reciprocal + DVE mult, bf16 tail
  - scatter-add: matmul(lhsT=msg[e,c], rhs=oh_scat[e,n] fp8) -> [c, n] in PSUM
  - MLP with global BatchNorm (stats allreduced); BN bias/scale folded so the
    h pass needs no mask; relu via DVE add+max with a_bn folded into W2
  - per-graph LayerNorm applied as one affine_mul_reduce per graph
  - AllGather of the bf16 node table for the next layer's gathers
"""
import sys

sys.path.insert(0, "/opt/trn_rl_repo")

import numpy as np
import ml_dtypes

N = 20000
E = 640000
C = 128
D = 64
H = 512
L = 3
G = 32
EPS = 1e-5
NCORES = 8
GPG = G // NCORES  # graphs per core = 4
SG = 6             # edge tiles per activation supergroup
SPL = 30           # edge tiles per gather split

BF16 = ml_dtypes.bfloat16
F8 = ml_dtypes.float8_e4m3


def _wrap_idx(idx):
    """[n] int -> [16, n//16] int16 in the gather engine's wrapped layout
    (replicated to 128 partitions on device)."""
    n = idx.shape[0]
    assert n % 16 == 0
    return np.ascontiguousarray(idx.reshape(n // 16, 16).T.astype(np.int16))


def _splits(T, lead=0):
    out = []
    t0 = 0
    if lead and T > lead + 4:
        out.append((0, lead))
        t0 = lead
    rest = T - t0
    nsp = (rest + SPL - 1) // SPL
    base = rest // nsp
    rem = rest % nsp
    for i in range(nsp):
        seg = base + (1 if i < rem else 0)
        out.append((t0, seg))
        t0 += seg
    return out


def _preprocess(x, node_batch, edge_index, edge_attr):
    nb = np.asarray(node_batch)
    ei = np.asarray(edge_index)
    NN = nb.shape[0]
    counts = np.bincount(nb, minlength=G)
    gstart = np.concatenate([[0], np.cumsum(counts)[:-1]])
    GPAD = max(128, int(np.ceil(counts.max() / 128)) * 128)
    NPAD = GPG * GPAD
    NB = NPAD // 128
    PTOT = NCORES * NPAD
    assert PTOT < 32768

    g_of = nb  # nodes sorted by graph
    core_of_node = g_of // GPG
    slot = (g_of % GPG) * GPAD + (np.arange(NN) - gstart[g_of])
    pad_slot = core_of_node * NPAD + slot  # global padded index

    src, dst = ei[0], ei[1]
    core_of_edge = core_of_node[dst]
    dst_local = pad_slot[dst] - core_of_edge * NPAD

    # per (core, block) edge lists
    order = np.lexsort((dst_local, core_of_edge))
    src_s, dst_s = src[order], dst_local[order]
    core_s = core_of_edge[order]
    blk_s = dst_s // 128
    cb = np.zeros((NCORES, NB), np.int64)
    np.add.at(cb, (core_s, blk_s), 1)
    T_b = [int(np.ceil(cb[:, b].max() / 128)) for b in range(NB)]
    TT = sum(T_b)
    EP = TT * 128

    core_edge_start = np.concatenate([[0], np.cumsum(np.bincount(core_s, minlength=NCORES))])
    per_core = []
    x_np = np.asarray(x)
    ea = np.asarray(edge_attr)

    for c in range(NCORES):
        lo, hi = core_edge_start[c], core_edge_start[c + 1]
        srcc, dstc, blkc = src_s[lo:hi], dst_s[lo:hi], blk_s[lo:hi]
        eidc = order[lo:hi]
        # slot edges into padded per-block tile space
        src_pad = np.zeros(EP, np.int64)
        dstrel = np.full(EP, -1, np.int64)
        attr_sel = np.zeros(EP, np.int64)
        attr_valid = np.zeros(EP, bool)
        off = 0
        boff = np.concatenate([[0], np.cumsum(np.bincount(blkc, minlength=NB))])
        for b in range(NB):
            cnt = boff[b + 1] - boff[b]
            sl = slice(boff[b], boff[b + 1])
            src_pad[off:off + cnt] = pad_slot[srcc[sl]]
            dstrel[off:off + cnt] = dstc[sl] - 128 * b
            attr_sel[off:off + cnt] = eidc[sl]
            attr_valid[off:off + cnt] = True
            off += T_b[b] * 128
        assert off == EP

        attrT = np.zeros((65, EP), BF16)
        attrT[:D, attr_valid] = ea[attr_sel[attr_valid]].astype(BF16).T
        attrT[64, :] = BF16(1.0)

        # one-hots (fp8, exact 0/1)
        epos = np.arange(EP)
        valid = dstrel >= 0
        oh_dst = np.zeros((128, EP), F8)
        oh_dst[dstrel[valid], epos[valid]] = F8(1.0)
        # oh_scat[p, t*128 + n] = 1 iff edge (tile t, slot p) has dstrel == n
        oh_scat = np.zeros((128, EP), F8)
        t_of = epos // 128
        p_of = epos % 128
        oh_scat[p_of[valid], t_of[valid] * 128 + dstrel[valid]] = F8(1.0)

        xcn = np.zeros((C, NPAD), np.float32)
        mask = np.zeros((1, NPAD), np.float32)
        own = core_of_node == c
        xcn[:, slot[own]] = x_np[own].T
        mask[0, slot[own]] = 1.0
        xncbf = np.zeros((NPAD, C), BF16)
        xncbf[slot[own]] = x_np[own].astype(BF16)
        invcnt = (1.0 / (np.maximum(counts[c * GPG:(c + 1) * GPG], 1) * C)).astype(np.float32)

        per_core.append(dict(
            srcidx=np.ascontiguousarray(np.tile(_wrap_idx(src_pad), (8, 1))),
            attrt=attrT,
            ohdst=oh_dst,
            ohsct=oh_scat,
            xcn=xcn,
            xbfc=xcn.astype(BF16),
            mask=np.ascontiguousarray(np.broadcast_to(mask, (128, NPAD))),
            invcntb=np.ascontiguousarray(np.broadcast_to(invcnt.reshape(1, GPG), (128, GPG))),
            xncbf=xncbf,
        ))

    meta = dict(GPAD=GPAD, NPAD=NPAD, NB=NB, PTOT=PTOT, T_b=T_b, TT=TT, EP=EP,
                pad_slot=pad_slot, counts=counts, NN=NN)
    return per_core, meta


def _prep_weights(Wf, bf, Ws, bs, W1, b1, g1, be1, W2, b2, lnw, lnb):
    # z-part weight blocks: columns 0:128 gate (negated for exp(-g)), 128:256 core
    wdst = np.zeros((L, 128, 256), np.float32)
    wsrc = np.zeros((L, 128, 256), np.float32)
    wa = np.zeros((L, 65, 256), np.float32)
    for l in range(L):
        wdst[l, :, 0:128] = -Wf[l][:, 0:C].T
        wdst[l, :, 128:256] = Ws[l][:, 0:C].T
        wsrc[l, :, 0:128] = -Wf[l][:, C:2 * C].T
        wsrc[l, :, 128:256] = Ws[l][:, C:2 * C].T
        wa[l, :D, 0:128] = -Wf[l][:, 2 * C:].T
        wa[l, :D, 128:256] = Ws[l][:, 2 * C:].T
        wa[l, 64, 0:128] = -bf[l]
        wa[l, 64, 128:256] = bs[l]
    w1t = np.stack([np.stack([W1[l].T[:, 128 * k:128 * (k + 1)] for k in range(4)]) for l in range(L)])
    w2t = np.stack([np.stack([W2[l].T[128 * k:128 * (k + 1), :] for k in range(4)]) for l in range(L)])
    b1c = np.asarray(b1, np.float32).reshape(L, 4, 128).transpose(2, 0, 1).reshape(128, L * 4)
    g1c = np.asarray(g1, np.float32).reshape(L, 4, 128).transpose(2, 0, 1).reshape(128, L * 4)
    be1c = np.asarray(be1, np.float32).reshape(L, 4, 128).transpose(2, 0, 1).reshape(128, L * 4)
    b2c = np.asarray(b2, np.float32).T
    lnwc = np.asarray(lnw, np.float32).T
    lnbc = np.asarray(lnb, np.float32).T
    consts = np.ascontiguousarray(np.concatenate(
        [b1c, g1c, be1c, b2c, lnwc, lnbc], axis=1))  # [128, 3*4L + 3*L]
    return dict(
        wdst=np.ascontiguousarray(wdst.transpose(1, 0, 2).reshape(128, L * 256)).astype(BF16),
        wsrc=np.ascontiguousarray(wsrc.transpose(1, 0, 2).reshape(128, L * 256)).astype(BF16),
        wa=np.ascontiguousarray(wa.transpose(1, 0, 2).reshape(65, L * 256)).astype(BF16),
        w1t=np.ascontiguousarray(w1t.reshape(L * 4, 128, 128).transpose(1, 0, 2).reshape(128, L * 4 * 128)).astype(BF16),
        w2t=np.ascontiguousarray(w2t.reshape(L * 4, 128, 128).transpose(1, 0, 2).reshape(128, L * 4 * 128)).astype(BF16),
        consts=consts,
    )


def _trace(meta, nlayers=L, use_cc=True, debug_stage=None):
    from concourse import bacc, mybir, bass_isa
    import concourse.tile as tile

    # Force every activation onto the exp+ln table (index 6) so the ACT
    # table-load pass never alternates tables between Exp and Ln ops.
    if not hasattr(bacc, "_orig_get_act_tables"):
        bacc._orig_get_act_tables = bacc.get_activation_tables

        def _only_table6(arch):
            tabs = bacc._orig_get_act_tables(arch)
            out = {}
            for name, funcs in tabs.items():
                out[name] = funcs if name == "natural_log_exp_and_others" else set()
            return out

        bacc.get_activation_tables = _only_table6

    F32 = mybir.dt.float32
    B16 = mybir.dt.bfloat16
    F8E4 = mybir.dt.float8e4
    I16 = mybir.dt.int16
    AF = mybir.ActivationFunctionType
    OP = mybir.AluOpType

    NPAD, NB, PTOT, TT, EP = meta["NPAD"], meta["NB"], meta["PTOT"], meta["TT"], meta["EP"]
    GPAD = meta["GPAD"]
    T_b = meta["T_b"]
    NN = meta["NN"]

    nc = bacc.Bacc("TRN2", target_bir_lowering=False, debug=False, num_devices=NCORES)

    def din(name, shape, dt):
        return nc.dram_tensor(name, shape, dt, kind="ExternalInput").ap()

    xcn_in = din("xcn", [C, NPAD], F32)
    xbfc_in = din("xbfc", [C, NPAD], B16)
    xncbf_in = din("xncbf", [NPAD, C], B16)
    srcidx_in = din("srcidx", [128, TT * 8], I16)
    attrt_in = din("attrt", [65, EP], B16)
    ohdst_in = din("ohdst", [128, EP], F8E4)
    ohsct_in = din("ohsct", [128, EP], F8E4)
    mask_in = din("mask", [128, NPAD], F32)
    invcntb_in = din("invcntb", [128, GPG], F32)
    ident_in = din("ident", [128, 128], F32)
    identb_in = din("identb", [128, 128], B16)
    wdst_in = din("wdst", [128, L * 256], B16)
    wsrc_in = din("wsrc", [128, L * 256], B16)
    wa_in = din("wa", [65, L * 256], B16)
    w1t_in = din("w1t", [128, L * 4 * 128], B16)
    w2t_in = din("w2t", [128, L * 4 * 128], B16)
    consts_in = din("consts", [128, 15 * L], F32)
    xout = nc.dram_tensor("xout", [NPAD, C], F32, kind="ExternalOutput").ap()

    splits = _splits

    with tile.TileContext(nc) as tc:
        with (
            tc.tile_pool(name="const", bufs=1) as cp,
            tc.tile_pool(name="xstate", bufs=2) as xp,
            tc.tile_pool(name="dram", bufs=1, space="DRAM") as dr,
        ):
            # ---- constants (critical-path loads first) ----
            tab0_b = dr.tile([NPAD, C], B16, tag="tab0b")
            nc.sync.dma_start(out=tab0_b[:], in_=xncbf_in[:])
            srcidx = cp.tile([128, TT * 8], I16)
            nc.sync.dma_start(out=srcidx[:], in_=srcidx_in[:])
            x_bf = xp.tile([C, NPAD], B16, tag="xbf", bufs=1, name="xbf")
            nc.sync.dma_start(out=x_bf[:], in_=xbfc_in[:])
            wdst = cp.tile([128, L, 256], B16)
            wsrc = cp.tile([128, L, 256], B16)
            wa = cp.tile([65, L, 256], B16)
            nc.sync.dma_start(out=wdst[:], in_=wdst_in[:])
            nc.sync.dma_start(out=wsrc[:], in_=wsrc_in[:])
            nc.sync.dma_start(out=wa[:], in_=wa_in[:])
            xtab_start = dr.tile([PTOT, C], B16, tag="xtab0", addr_space="Shared")
            if use_cc:
                nc.gpsimd.collective_compute(
                    "AllGather", OP.bypass, replica_groups=[list(range(NCORES))],
                    ins=[tab0_b[:].opt()], outs=[xtab_start[:].opt()])
            else:
                nc.sync.dma_start(out=xtab_start[0:NPAD, :], in_=tab0_b[:])

            x_fp = xp.tile([C, NPAD], F32, tag="xfp", bufs=1, name="xfp")
            nc.scalar.dma_start(out=x_fp[:], in_=xcn_in[:])
            U_all = xp.tile([128, NB, 256], B16, tag="uall", bufs=1, name="uall")
            mask = cp.tile([128, NPAD], F32)
            nc.scalar.dma_start(out=mask[:], in_=mask_in[:])
            invcntb = cp.tile([128, GPG], F32)
            nc.scalar.dma_start(out=invcntb[:], in_=invcntb_in[:])
            ident = cp.tile([128, 128], F32)
            nc.scalar.dma_start(out=ident[:], in_=ident_in[:])
            identb = cp.tile([128, 128], B16)
            nc.scalar.dma_start(out=identb[:], in_=identb_in[:])
            w1t = cp.tile([128, L * 4, 128], B16)
            w2t = cp.tile([128, L * 4, 128], B16)
            nc.scalar.dma_start(out=w1t[:], in_=w1t_in[:])
            nc.scalar.dma_start(out=w2t[:], in_=w2t_in[:])
            allc = cp.tile([128, 15 * L], F32)
            nc.scalar.dma_start(out=allc[:], in_=consts_in[:])
            b1c = allc[:, 0:4 * L]
            g1c = allc[:, 4 * L:8 * L]
            be1c = allc[:, 8 * L:12 * L]
            b2c = allc[:, 12 * L:13 * L]
            lnwc = allc[:, 13 * L:14 * L]
            lnbc = allc[:, 14 * L:15 * L]
            epsc = cp.tile([128, 1], F32)
            nc.gpsimd.memset(epsc[:], EPS)
            agouts = []
            agins = []
            for l in range(nlayers):
                tab = xtab_start[:] if l == 0 else agouts[l - 1][:]
                if l == 0:
                    # dst-side preacts per block (later layers fold this into
                    # the previous node phase, per graph)
                    with tc.tile_pool(name="upsum", bufs=2, space="PSUM") as up:
                        for b in range(NB):
                            ups = up.tile([128, 256], F32, tag="ups", space="PSUM")
                            nc.tensor.matmul(out=ups[:], lhsT=x_bf[:, 128 * b:128 * (b + 1)],
                                             rhs=wdst[:, l, :], start=True, stop=True)
                            nc.vector.tensor_scalar(out=U_all[:, b, :], in0=ups[:],
                                                    scalar1=1.0, scalar2=0.0, op0=OP.mult, op1=OP.add)
                # ---------------- edge phase ----------------
                x2_fp = xp.tile([C, NPAD], F32, tag="x2fp", bufs=1, name=f"x2fp_{l}")
                x2_bf = xp.tile([C, NPAD], B16, tag="x2bf", bufs=1, name=f"x2bf_{l}")
                with (
                    tc.tile_pool(name="egath", bufs=3) as gp,
                    tc.tile_pool(name="eact", bufs=2) as ep,
                    tc.tile_pool(name="epsum", bufs=2, space="PSUM") as pp,
                    tc.tile_pool(name="aggpsum", bufs=2, space="PSUM") as ap_,
                ):
                    toff = 0
                    for b in range(NB):
                        T = T_b[b]
                        bcol = slice(128 * b, 128 * (b + 1))
                        if T == 0:
                            nc.vector.tensor_copy(x2_fp[:, bcol], x_fp[:, bcol])
                            nc.vector.tensor_copy(x2_bf[:, bcol], x_fp[:, bcol])
                            continue
                        agg = ap_.tile([C, 128], F32, tag="agg", space="PSUM")
                        for (ts0, seg) in splits(T):
                            ni = seg * 128
                            e0 = (toff + ts0) * 128
                            zsrc = gp.tile([128, 1, ni], B16, tag="zsrc")
                            nc.gpsimd.dma_gather(zsrc[:], tab, srcidx[:, (toff + ts0) * 8:(toff + ts0 + seg) * 8],
                                                 num_idxs=ni, num_idxs_reg=ni, elem_size=C,
                                                 transpose=True, single_packet=False)
                            attr = gp.tile([65, ni], B16, tag="attr")
                            nc.sync.dma_start(out=attr[:], in_=attrt_in[:, e0:e0 + ni])
                            ohd = gp.tile([128, ni], F8E4, tag="ohd")
                            nc.sync.dma_start(out=ohd[:], in_=ohdst_in[:, e0:e0 + ni])
                            ohs = gp.tile([128, ni], F8E4, tag="ohs")
                            nc.sync.dma_start(out=ohs[:], in_=ohsct_in[:, e0:e0 + ni])
                            t0 = ts0
                            while t0 < ts0 + seg:
                                sgn = min(SG, ts0 + seg - t0)
                                pre = pp.tile([128, SG, 256], F32, tag="pre", space="PSUM")
                                for t in range(t0, t0 + sgn):
                                    s = t - t0
                                    esl = slice((t - ts0) * 128, (t - ts0 + 1) * 128)
                                    nc.tensor.matmul(out=pre[:, s, :], lhsT=ohd[:, esl],
                                                     rhs=U_all[:, b, :], start=True, stop=False)
                                    nc.tensor.matmul(out=pre[:, s, :], lhsT=zsrc[:, 0, esl],
                                                     rhs=wsrc[:, l, :], start=False, stop=False)
                                    nc.tensor.matmul(out=pre[:, s, :], lhsT=attr[0:65, esl],
                                                     rhs=wa[0:65, l, :], start=False, stop=True)
                                uv = ep.tile([128, SG, 256], B16, tag="uv")
                                t32 = ep.tile([128, SG, 128], F32, tag="t32")
                                sp = ep.tile([128, SG, 128], B16, tag="sp")
                                r32 = ep.tile([128, SG, 128], F32, tag="r32")
                                msg = ep.tile([128, SG, 128], B16, tag="msg")
                                nc.scalar.activation(uv[:, :sgn, :], pre[:, :sgn, :], AF.Exp)
                                nc.scalar.activation(sp[:, :sgn, :], uv[:, :sgn, C:2 * C], AF.Ln, bias=1.0)
                                nc.gpsimd.tensor_scalar_add(t32[:, :sgn, :], uv[:, :sgn, 0:C], 1.0)
                                nc.vector.reciprocal_approx_fast(out=r32[:, :sgn, :], in_=t32[:, :sgn, :])
                                nc.vector.tensor_tensor(out=msg[:, :sgn, :], in0=sp[:, :sgn, :],
                                                        in1=r32[:, :sgn, :], op=OP.mult)
                                for t in range(t0, t0 + sgn):
                                    s = t - t0
                                    nsl = slice((t - ts0) * 128, (t - ts0 + 1) * 128)
                                    nc.tensor.matmul(out=agg[:], lhsT=msg[:, s, :], rhs=ohs[:, nsl],
                                                     start=(t == 0), stop=(t == T - 1))
                                t0 += sgn
                        nc.vector.tensor_tensor(out=x2_fp[:, bcol], in0=x_fp[:, bcol], in1=agg[:], op=OP.add)
                        nc.vector.tensor_scalar(out=x2_bf[:, bcol], in0=x2_fp[:, bcol],
                                                scalar1=1.0, scalar2=0.0, op0=OP.mult, op1=OP.add)
                        toff += T

                if debug_stage == "x2":
                    with tc.tile_pool(name="dbg", bufs=2, space="PSUM") as dbp:
                        with tc.tile_pool(name="dbw", bufs=2) as dbw:
                            for b in range(NB):
                                tpd = dbp.tile([128, 128], F32, tag="dtp", space="PSUM")
                                nc.tensor.transpose(out=tpd[:], in_=x2_fp[:, 128 * b:128 * (b + 1)],
                                                    identity=ident[:])
                                xo = dbw.tile([128, 128], F32, tag="dxo")
                                nc.vector.tensor_copy(xo[:], tpd[:])
                                nc.sync.dma_start(out=xout[128 * b:128 * (b + 1), :], in_=xo[:])
                    break
                # ---------------- node phase ----------------
                with tc.tile_pool(name="nsb", bufs=1) as np_:
                    h_bf = np_.tile([128, 4, NPAD], B16)
                    s1p = np_.tile([128, 4, GPG], F32)
                    s2p = np_.tile([128, 4, GPG], F32)
                    with (
                        tc.tile_pool(name="hwork", bufs=2) as hw,
                        tc.tile_pool(name="hpsum", bufs=2, space="PSUM") as hp_,
                    ):
                        for g in range(GPG):
                            glo = g * GPAD
                            gsl = slice(glo, glo + GPAD)
                            for k in range(4):
                                hp = hp_.tile([128, GPAD], F32, tag="hp", space="PSUM")
                                for mlo in range(0, GPAD, 512):
                                    w = min(512, GPAD - mlo)
                                    nc.tensor.matmul(out=hp[:, mlo:mlo + w], lhsT=w1t[:, 4 * l + k, :],
                                                     rhs=x2_bf[:, glo + mlo:glo + mlo + w], start=True, stop=True)
                                # h (no bias; padded cols exactly 0) + rowsum on Pool
                                junk = hw.tile([128, GPAD], B16, tag="junk")
                                if k % 2 == 0:
                                    nc.vector.tensor_scalar(out=h_bf[:, k, gsl], in0=hp[:],
                                                            scalar1=1.0, scalar2=0.0, op0=OP.mult, op1=OP.add,
                                                            accum_out=s1p[:, k, g:g + 1])
                                    nc.scalar.activation(junk[:], h_bf[:, k, gsl], AF.Square,
                                                         accum_out=s2p[:, k, g:g + 1])
                                else:
                                    nc.scalar.activation(h_bf[:, k, gsl], hp[:], AF.Copy,
                                                         accum_out=s1p[:, k, g:g + 1])
                                    nc.vector.scalar_tensor_tensor(out=junk[:], in0=h_bf[:, k, gsl],
                                                                   scalar=0.0, in1=h_bf[:, k, gsl],
                                                                   op0=OP.add, op1=OP.mult,
                                                                   accum_out=s2p[:, k, g:g + 1])
                    bnstat = np_.tile([128, 8], F32)
                    nc.vector.tensor_reduce(out=bnstat[:, 0:4], in_=s1p[:], axis=mybir.AxisListType.X, op=OP.add)
                    nc.vector.tensor_reduce(out=bnstat[:, 4:8], in_=s2p[:], axis=mybir.AxisListType.X, op=OP.add)
                    bnin = dr.tile([128, 8], F32, tag="bnin", bufs=2)
                    bnout = dr.tile([128, 8], F32, tag="bnout", bufs=2, addr_space="Shared")
                    nc.sync.dma_start(out=bnin[:], in_=bnstat[:])
                    bns = np_.tile([128, 8], F32)
                    if use_cc:
                        nc.gpsimd.collective_compute(
                            "AllReduce", OP.add, replica_groups=[list(range(NCORES))],
                            ins=[bnin[:].opt()], outs=[bnout[:].opt()])
                        nc.sync.dma_start(out=bns[:], in_=bnout[:])
                    else:
                        nc.vector.tensor_scalar(out=bns[:], in0=bnstat[:], scalar1=float(NCORES),
                                                scalar2=None, op0=OP.mult)
                    ksl = slice(4 * l, 4 * l + 4)
                    mean_r = np_.tile([128, 4], F32)
                    nc.vector.tensor_scalar(out=mean_r[:], in0=bns[:, 0:4], scalar1=1.0 / NN,
                                            scalar2=None, op0=OP.mult)
                    var = np_.tile([128, 4], F32)
                    nc.vector.tensor_scalar(out=var[:], in0=bns[:, 4:8], scalar1=1.0 / NN,
                                            scalar2=None, op0=OP.mult)
                    msq = np_.tile([128, 4], F32)
                    nc.vector.tensor_tensor(out=msq[:], in0=mean_r[:], in1=mean_r[:], op=OP.mult)
                    nc.vector.tensor_tensor(out=var[:], in0=var[:], in1=msq[:], op=OP.subtract)
                    rstd = np_.tile([128, 4], F32)
                    nc.scalar.activation(rstd[:], var[:], AF.Ln, bias=epsc[:])
                    nc.scalar.activation(rstd[:], rstd[:], AF.Exp, scale=-0.5)
                    a_bn = np_.tile([128, 4], F32)
                    nc.vector.tensor_tensor(out=a_bn[:], in0=rstd[:], in1=g1c[:, ksl], op=OP.mult)
                    inva = np_.tile([128, 4], F32)
                    nc.vector.reciprocal_approx_fast(out=inva[:], in_=a_bn[:])
                    # q = be1/a + b1 - (mean_r + b1) = be1/a - mean_r
                    q = np_.tile([128, 4], F32)
                    nc.vector.tensor_tensor(out=q[:], in0=be1c[:, ksl], in1=inva[:], op=OP.mult)
                    nc.vector.tensor_tensor(out=q[:], in0=q[:], in1=mean_r[:], op=OP.subtract)
                    w2s = np_.tile([128, 4, 128], B16)
                    for k in range(4):
                        nc.vector.tensor_scalar(out=w2s[:, k, :], in0=w2t[:, 4 * l + k, :],
                                                scalar1=a_bn[:, k:k + 1], scalar2=0.0,
                                                op0=OP.mult, op1=OP.add)

                    if l < nlayers - 1:
                        agin = dr.tile([NPAD, C], B16, tag="agin", bufs=2)
                        if use_cc:
                            agout = dr.tile([PTOT, C], B16, tag="agout", bufs=2, addr_space="Shared")
                        else:
                            agout = dr.tile([PTOT, C], B16, tag="agout", bufs=2)
                        agins.append(agin)
                    y_fp = xp.tile([C, NPAD], F32, tag="xfp", bufs=1, name=f"yfp_{l}")
                    y_bf = xp.tile([C, NPAD], B16, tag="xbf", bufs=1, name=f"ybf_{l}")
                    with (
                        tc.tile_pool(name="xwork", bufs=3) as nw,
                        tc.tile_pool(name="xpsum", bufs=2, space="PSUM") as xpp_,
                        tc.tile_pool(name="tpsum", bufs=2, space="PSUM") as tp_,
                    ):
                        for g in range(GPG):
                            glo = g * GPAD
                            gsl = slice(glo, glo + GPAD)
                            xpp = xpp_.tile([128, GPAD], F32, tag="xpp", space="PSUM")
                            for k in range(4):
                                hn = nw.tile([128, GPAD], B16, tag="hn")
                                nc.vector.tensor_scalar(out=hn[:], in0=h_bf[:, k, gsl],
                                                        scalar1=q[:, k:k + 1], scalar2=0.0,
                                                        op0=OP.add, op1=OP.max)
                                for mlo in range(0, GPAD, 512):
                                    w = min(512, GPAD - mlo)
                                    nc.tensor.matmul(out=xpp[:, mlo:mlo + w], lhsT=w2s[:, k, :],
                                                     rhs=hn[:, mlo:mlo + w], start=(k == 0), stop=False)
                            for mlo in range(0, GPAD, 512):
                                w = min(512, GPAD - mlo)
                                nc.tensor.matmul(out=xpp[:, mlo:mlo + w], lhsT=identb[:],
                                                 rhs=x2_bf[:, glo + mlo:glo + mlo + w],
                                                 start=False, stop=True)
                            # x3 = (xpp + b2) * mask, with row-sums for LN
                            x3m = nw.tile([128, GPAD], F32, tag="x3m")
                            ls = np_.tile([128, 2], F32, tag="ls", bufs=4)
                            nc.vector.affine_mul_reduce(out=x3m[:], accum_out=ls[:, 0:1],
                                                        in0=xpp[:], in1=mask[:, gsl],
                                                        scale=1.0, bias=b2c[:, l:l + 1])
                            junk2 = nw.tile([128, GPAD], B16, tag="junk2")
                            nc.scalar.activation(junk2[:], x3m[:], AF.Square,
                                                 accum_out=ls[:, 1:2])
                            if debug_stage == "x3":
                                for bb in range(GPAD // 128):
                                    gb = glo + 128 * bb
                                    tpd = tp_.tile([128, 128], F32, tag="dtp", space="PSUM")
                                    nc.tensor.transpose(out=tpd[:], in_=x3m[:, 128 * bb:128 * (bb + 1)],
                                                        identity=ident[:])
                                    xo = nw.tile([128, 128], F32, tag="dxo")
                                    nc.vector.tensor_copy(xo[:], tpd[:])
                                    nc.sync.dma_start(out=xout[gb:gb + 128, :], in_=xo[:])
                                continue
                            lsr = np_.tile([128, 2], F32, tag="lsr", bufs=4)
                            nc.gpsimd.partition_all_reduce(lsr[:], ls[:], channels=128,
                                                           reduce_op=bass_isa.ReduceOp.add)
                            mv = np_.tile([128, 2], F32, tag="mv", bufs=4)
                            nc.vector.tensor_scalar(out=mv[:], in0=lsr[:], scalar1=invcntb[:, g:g + 1],
                                                    scalar2=None, op0=OP.mult)
                            if debug_stage == "lnstats":
                                nc.sync.dma_start(out=xout[glo:glo + 1, 0:2], in_=ls[0:1, :])
                                nc.sync.dma_start(out=xout[glo + 1:glo + 2, 0:2], in_=lsr[0:1, :])
                                nc.sync.dma_start(out=xout[glo + 2:glo + 3, 0:2], in_=mv[0:1, :])
                                continue
                            m2g = np_.tile([128, 1], F32, tag="m2g", bufs=2)
                            nc.vector.tensor_tensor(out=m2g[:], in0=mv[:, 0:1], in1=mv[:, 0:1], op=OP.mult)
                            vgg = np_.tile([128, 1], F32, tag="vgg", bufs=2)
                            nc.vector.tensor_tensor(out=vgg[:], in0=mv[:, 1:2], in1=m2g[:], op=OP.subtract)
                            rgg = np_.tile([128, 1], F32, tag="rgg", bufs=2)
                            nc.scalar.activation(rgg[:], vgg[:], AF.Ln, bias=epsc[:])
                            nc.scalar.activation(rgg[:], rgg[:], AF.Exp, scale=-0.5)
                            scal = np_.tile([128, 1], F32, tag="scal", bufs=2)
                            nc.vector.tensor_tensor(out=scal[:], in0=lnwc[:, l:l + 1], in1=rgg[:], op=OP.mult)
                            bias = np_.tile([128, 1], F32, tag="bias", bufs=2)
                            nc.vector.tensor_tensor(out=bias[:], in0=mv[:, 0:1], in1=scal[:], op=OP.mult)
                            nc.vector.tensor_tensor(out=bias[:], in0=lnbc[:, l:l + 1], in1=bias[:], op=OP.subtract)
                            # y = (x3m * scal + bias) * mask  (pads stay exactly 0)
                            jacc = np_.tile([128, 1], F32, tag="jacc", bufs=2)
                            nc.vector.affine_mul_reduce(out=y_fp[:, gsl], accum_out=jacc[:],
                                                        in0=x3m[:], in1=mask[:, gsl],
                                                        scale=scal[:], bias=bias[:])
                            nc.vector.tensor_scalar(out=y_bf[:, gsl], in0=y_fp[:, gsl],
                                                    scalar1=1.0, scalar2=0.0, op0=OP.mult, op1=OP.add)
                            # ship this graph's blocks: batched transposes,
                            # one copy, one DMA
                            NBG = GPAD // 128
                            if l < nlayers - 1:
                                xnc = nw.tile([128, NBG, 128], B16, tag="xnc")
                                for bb in range(NBG):
                                    gb = glo + 128 * bb
                                    tp = tp_.tile([128, 128], B16, tag="tp", space="PSUM", bufs=2)
                                    nc.tensor.transpose(out=tp[:], in_=y_bf[:, gb:gb + 128],
                                                        identity=identb[:])
                                    if bb % 2 == 0:
                                        nc.scalar.activation(xnc[:, bb, :], tp[:], AF.Copy)
                                    else:
                                        nc.vector.tensor_scalar(out=xnc[:, bb, :], in0=tp[:], scalar1=1.0,
                                                                scalar2=0.0, op0=OP.mult, op1=OP.add)
                                    nc.sync.dma_start(out=agin[gb:gb + 128, :], in_=xnc[:, bb, :])
                            else:
                                xnc32 = nw.tile([128, NBG, 128], F32, tag="xnc32")
                                for bb in range(NBG):
                                    gb = glo + 128 * bb
                                    tp = tp_.tile([128, 128], F32, tag="tpf", space="PSUM", bufs=2)
                                    nc.tensor.transpose(out=tp[:], in_=y_fp[:, gb:gb + 128],
                                                        identity=ident[:])
                                    if bb % 2 == 0:
                                        nc.scalar.activation(xnc32[:, bb, :], tp[:], AF.Copy)
                                    else:
                                        nc.vector.tensor_scalar(out=xnc32[:, bb, :], in0=tp[:], scalar1=1.0,
                                                                scalar2=0.0, op0=OP.mult, op1=OP.add)
                                    nc.sync.dma_start(out=xout[gb:gb + 128, :], in_=xnc32[:, bb, :])
                            if l < nlayers - 1:
                                # next layer's dst-side preacts for this graph's blocks
                                for bb in range(GPAD // 128):
                                    gb = glo + 128 * bb
                                    b = gb // 128
                                    ups = tp_.tile([128, 256], F32, tag="ups", space="PSUM")
                                    nc.tensor.matmul(out=ups[:], lhsT=y_bf[:, gb:gb + 128],
                                                     rhs=wdst[:, l + 1, :], start=True, stop=True)
                                    if bb % 2 == 0:
                                        nc.vector.tensor_scalar(out=U_all[:, b, :], in0=ups[:],
                                                                scalar1=1.0, scalar2=0.0, op0=OP.mult, op1=OP.add)
                                    else:
                                        nc.scalar.activation(U_all[:, b, :], ups[:], AF.Copy)
                                if not use_cc:
                                    nc.sync.dma_start(out=agout[glo:glo + GPAD, :],
                                                      in_=agin[glo:glo + GPAD, :])
                    if l < nlayers - 1:
                        agouts.append(agout)
                        if use_cc:
                            nc.gpsimd.collective_compute(
                                "AllGather", OP.bypass, replica_groups=[list(range(NCORES))],
                                ins=[agin[:].opt()], outs=[agout[:].opt()])
                x_fp = y_fp
                x_bf = y_bf

    nc.finalize()
    return nc


_CACHE = {}


def kernel(x, node_batch, edge_index, edge_attr,
           Wf, bf, Ws, bs, W1, b1, g1, be1, W2, b2, lnw, lnb):
    from concourse.bass_utils import run_bass_kernel_spmd

    per_core, meta = _preprocess(x, node_batch, edge_index, edge_attr)
    wd = _prep_weights(Wf, bf, Ws, bs, W1, b1, g1, be1, W2, b2, lnw, lnb)
    key = (meta["NPAD"], meta["NN"], tuple(meta["T_b"]))
    if key not in _CACHE:
        _CACHE[key] = _trace(meta)
    nc = _CACHE[key]

    ident = np.eye(128, dtype=np.float32)
    identb = np.eye(128, dtype=np.float32).astype(BF16)
    in_maps = []
    for c in range(NCORES):
        m = dict(per_core[c])
        m.update(wd)
        m.update(ident=ident, identb=identb)
        in_maps.append(m)
    res = run_bass_kernel_spmd(nc, in_maps, list(range(NCORES)))

    pad_slot = meta["pad_slot"]
    NPAD = meta["NPAD"]
    out = np.zeros((meta["NN"], C), np.float32)
    for c in range(NCORES):
        own = (pad_slot >= c * NPAD) & (pad_slot < (c + 1) * NPAD)
        out[own] = res.results[c]["xout"][pad_slot[own] - c * NPAD]
    return out
